# revision 40
# baseline (speedup 1.0000x reference)
# Bass/Trainium2 kernel for nn_Delta (DeltaNet-style recurrence).
#
# Problem (hardcoded): N=8, T=2048, C=512, fp32 I/O.
#   g = x @ Wg.T + bg ; q,k,v = split(g) ; lr = x @ Wlr.T + blr
#   khat = k / ||k||
#   delta-rule scan:  u_t = v_t - khat_t @ S ; S += outer(khat_t, u_t)
#   kv = sum_t khat_t (x) (lr_t * u_t) ; y = q @ kv ; out = y @ Wo.T + bo
#
# Sharding: data-parallel over N across the 8 cores (sample i -> core i),
# weights replicated. No collectives.
#
# Per-core algorithm: chunked parallel delta rule, chunk L=128, in
# "W-space": with D = diag(1/||k_raw||) per chunk and W = D^-1-free
# substitution W = D u, the recurrence becomes
#   (I + D^2 G) W = D V - D^2 (Kraw S + cross),   G = tril(Kraw Kraw^T, -1)
#   S += Kraw^T W,   kv = Kraw^T (lr . W)
# so only RAW projections appear in matmuls; the 1/||k|| and 1/||k||^2
# row scalings ride on PSUM evacuations (per-partition scale APs).
#
# (I + B)^-1 with B = -D^2 G is truncated exactly through degree 11 via
# the two-factor form (I+F1)(I+F2), F1 = B+B^2+B^3, F2 = B^4+B^8,
# applied merged: W = rhs + M' rhs with M' = F1+F2+F1F2  (measured
# truncation error 2.3e-3 in f64 on this data, far below the bf16 noise).
#
# Q is never materialized: y = q @ kv folds into out = x @ R + brow with
# R = Wgq^T (kv Wo^T) and brow = bgq (kv Wo^T) + bo, reusing resident xT.
#
# No DMA transposes: x, Wg, Wo are cast-DMA'd naturally and transposed
# on the tensor engine (transpose mode), as is Bl -> Bu.

import os

import numpy as np

N, T, C = 8, 2048, 512
L = 128
NCH = T // L  # 16 chunks
NP = NCH // 2  # 8 chunk pairs
CT = C // 128  # 4 c-tiles

_CACHE = {}


def _build():
    import concourse.bacc as bacc
    import concourse.mybir as mybir
    import concourse.tile as tile
    from concourse.bass import ts, ds
    from concourse.masks import make_identity, make_lower_triangular

    f32 = mybir.dt.float32
    bf16 = mybir.dt.bfloat16
    AF = mybir.ActivationFunctionType
    OP = mybir.AluOpType

    nc = bacc.Bacc("TRN2")
    x = nc.declare_dram_parameter("x", [T, C], f32, isOutput=False)
    Wg = nc.declare_dram_parameter("Wg", [3 * C, C], f32, isOutput=False)
    bg = nc.declare_dram_parameter("bg", [3 * C], f32, isOutput=False)
    Wlr = nc.declare_dram_parameter("Wlr", [1, C], f32, isOutput=False)
    blr = nc.declare_dram_parameter("blr", [1], f32, isOutput=False)
    Wo = nc.declare_dram_parameter("Wo", [C, C], f32, isOutput=False)
    bo = nc.declare_dram_parameter("bo", [C], f32, isOutput=False)
    out = nc.declare_dram_parameter("out", [T, C], f32, isOutput=True)
    dbg = os.environ.get("KDBG") == "1"
    if dbg:
        dbg_W = nc.declare_dram_parameter("dbg_W", [T, C], f32, isOutput=True)
        dbg_K = nc.declare_dram_parameter("dbg_K", [T, C], f32, isOutput=True)
        dbg_S = nc.declare_dram_parameter("dbg_S", [C, C], f32, isOutput=True)
        dbg_KVT = nc.declare_dram_parameter("dbg_KVT", [C, C], f32, isOutput=True)

    mm = nc.tensor.matmul

    with tile.TileContext(nc) as tc:
        with tc.tile_pool(name="persist", bufs=1) as P:
            # ---- constants / small tensors ----
            maskLn = P.tile([128, 128], f32, name="maskLn")
            make_lower_triangular(nc, maskLn[:], val=-1.0, diag=False)
            ones_bf = P.tile([1, 128], bf16, name="ones_bf")
            nc.vector.memset(ones_bf[:], 1.0)
            ident_bf = P.tile([128, 128], bf16, name="ident_bf")
            make_identity(nc, ident_bf[:])

            bgk_sb = P.tile([1, C], bf16, name="bgk_sb")
            bgv_sb = P.tile([1, C], bf16, name="bgv_sb")
            bo_sb = P.tile([1, C], bf16, name="bo_sb")
            bgq_sb = P.tile([128, CT], bf16, name="bgq_sb")
            WlrT_sb = P.tile([128, CT], bf16, name="WlrT_sb")
            blr_sb = P.tile([1, 1], f32, name="blr_sb")

            def load_small():
                nc.gpsimd.dma_start(out=bgk_sb[:], in_=bg[C:2 * C])
                nc.gpsimd.dma_start(out=bgv_sb[:], in_=bg[2 * C:3 * C])
                nc.gpsimd.dma_start(out=bo_sb[:], in_=bo[:])
                nc.gpsimd.dma_start(
                    out=bgq_sb[:], in_=bg[0:C].rearrange("(i p) -> p i", p=128)
                )
                nc.gpsimd.dma_start(
                    out=WlrT_sb[:], in_=Wlr[0, :].rearrange("(i p) -> p i", p=128)
                )
                nc.gpsimd.dma_start(out=blr_sb[:], in_=blr[:])

            # ---- persistent tensors ----
            xT = [P.tile([128, T], bf16, name=f"xT{i}") for i in range(CT)]
            WgT = [P.tile([128, 3 * C], bf16, name=f"WgT{i}") for i in range(CT)]
            WoT = [P.tile([128, C], bf16, name=f"WoT{i}") for i in range(CT)]
            KTr = [P.tile([128, T], bf16, name=f"KTr{i}") for i in range(CT)]
            Kn = [P.tile([128, C], bf16, name=f"Kn{i}") for i in range(NCH)]
            Wt = [P.tile([128, C], bf16, name=f"Wt{i}") for i in range(NCH)]
            lru = [P.tile([128, C], bf16, name=f"lru{i}") for i in range(NCH)]
            Wq = [P.tile([128, C], bf16, name=f"Wq{i}") for i in range(CT)]
            Rt = [P.tile([128, C], bf16, name=f"Rt{i}") for i in range(CT)]
            bo2_b = P.tile([128, C], f32, name="bo2_b")
            S_f32 = [P.tile([128, C], f32, name=f"Sf{i}") for i in range(CT)]
            S_sb = [P.tile([128, C], bf16, name=f"S{i}") for i in range(CT)]
            KVT = [P.tile([128, C], bf16, name=f"KVT{i}") for i in range(CT)]
            KVW = [P.tile([128, C], bf16, name=f"KVW{i}") for i in range(CT)]
            bgk_b = P.tile([128, C], bf16, name="bgk_b")
            bo_b = P.tile([128, C], f32, name="bo_b")
            lrT = P.tile([1, T], f32, name="lrT")
            lrn = P.tile([128, NCH], f32, name="lrn")
            n2_all = P.tile([128, NCH], f32, name="n2_all")
            rn2 = P.tile([128, NCH], f32, name="rn2")  # 1/||k||^2
            rn2n = P.tile([128, NCH], f32, name="rn2n")  # -1/||k||^2
            rn_all = P.tile([128, NCH], f32, name="rn_all")  # 1/||k||
            # per-chunk construction outputs (consumed next pair at latest)
            Mu = [P.tile([128, 128], bf16, name=f"Mu{i}") for i in range(NCH)]
            GX = [P.tile([128, 128], bf16, name=f"GX{i}") for i in range(NP)]

            # ============ phase A+B: loads, PE transposes, projections =======
            # Load order: x, Wg-k block, then the kps/KT chain runs on PE
            # while Wg-v, Wg-q, Wo stream in behind it.  Wg-q row tiles are
            # also kept in natural layout (Wq) for the output-side fold
            # y = x @ (Wgq^T kv Wo^T): Q is never materialized.
            with tc.tile_pool(name="stg", bufs=2) as STG, \
                 tc.tile_pool(name="sbB", bufs=4) as SBB, \
                 tc.tile_pool(name="psT", bufs=2, space="PSUM") as PST, \
                 tc.tile_pool(name="psB", bufs=2, space="PSUM") as PSB, \
                 tc.tile_pool(name="psKT", bufs=2, space="PSUM") as PSKT, \
                 tc.tile_pool(name="psL", bufs=1, space="PSUM") as PSL:

                def load_dma(src, row0):
                    grp = []
                    for jj in range(4):
                        t = STG.tile([128, C], bf16, name=f"stg{jj}")
                        nc.gpsimd.dma_start(
                            out=t[:],
                            in_=src[row0 + jj * 128:row0 + (jj + 1) * 128, :],
                        )
                        grp.append(t)
                    return grp

                def load_tp(grp, dstT, col0):
                    for ci in range(CT):
                        ps = PST.tile([128, 512], bf16, name="pst")
                        for jj in range(4):
                            nc.tensor.transpose(
                                ps[:, ts(jj, 128)],
                                grp[jj][:, ts(ci, 128)],
                                ident_bf[:],
                            )
                        nc.any.tensor_copy(dstT[ci][:, ds(col0, 512)], ps[:])

                def load_group(src, row0, dstT, col0):
                    load_tp(load_dma(src, row0), dstT, col0)

                load_group(x, 0, xT, 0)
                load_group(Wg, C, WgT, C)  # k rows
                load_small()
                load_group(x, 512, xT, 512)
                # bias broadcast rows -> [128, C] tiles (one matmul each)
                bps = PSL.tile([128, C], f32, name="bps")
                mm(bps[:], lhsT=ones_bf[:], rhs=bgk_sb[:], start=True, stop=True)
                nc.any.tensor_copy(bgk_b[:], bps[:])
                bps2 = PSL.tile([128, C], f32, name="bps")
                mm(bps2[:], lhsT=ones_bf[:], rhs=bo_sb[:], start=True, stop=True)
                nc.any.tensor_copy(bo_b[:], bps2[:])

                for tj in range(NCH):
                    if tj == 4:
                        load_group(x, 2 * 512, xT, 2 * 512)
                    elif tj == 6:
                        load_group(x, 3 * 512, xT, 3 * 512)
                    kps = PSB.tile([128, C], f32, name="kps")
                    for ci in range(CT):
                        mm(kps[:], lhsT=xT[ci][:, ts(tj, 128)],
                           rhs=WgT[ci][:, ds(C, C)],
                           start=(ci == 0), stop=(ci == 3))
                    # Kn = kps + bgk (broadcast tile); n2 = sum Kn^2
                    nc.vector.tensor_tensor(
                        Kn[tj][:], kps[:], bgk_b[:], OP.add
                    )
                    junk = SBB.tile([128, C], f32, name="junk")
                    nc.vector.scalar_tensor_tensor(
                        junk[:], Kn[tj][:], 1.0, Kn[tj][:], OP.mult, OP.mult,
                        accum_out=n2_all[:, tj:tj + 1],
                    )
                    if tj % 4 == 3:
                        for ci in range(CT):
                            ps = PSKT.tile([128, 512], bf16, name="pskt")
                            for jj in range(4):
                                nc.tensor.transpose(
                                    ps[:, ts(jj, 128)],
                                    Kn[tj - 3 + jj][:, ts(ci, 128)],
                                    ident_bf[:],
                                )
                            nc.any.tensor_copy(
                                KTr[ci][:, ds((tj - 3) * 128, 512)], ps[:]
                            )
                    if tj == 3:
                        load_group(Wg, 2 * C, WgT, 2 * C)  # v rows
                    elif tj == 7:
                        for jj in range(4):  # q rows, natural layout only
                            nc.gpsimd.dma_start(
                                out=Wq[jj][:], in_=Wg[jj * 128:(jj + 1) * 128, :]
                            )
                    elif tj == 11:
                        load_group(Wo, 0, WoT, 0)

                # row scalings
                nc.vector.reciprocal(rn2[:], n2_all[:])
                nc.vector.tensor_scalar_mul(rn2n[:], rn2[:], -1.0)
                nc.scalar.activation(rn_all[:], rn2[:], AF.Sqrt)

                # lr row: lrT[1, T] then scatter to lrn [128, NCH]
                for tg in range(4):
                    lps = PSL.tile([1, 512], f32, name="lps")
                    for ci in range(CT):
                        mm(lps[:], lhsT=WlrT_sb[:, ci:ci + 1],
                           rhs=xT[ci][:, ds(tg * 512, 512)],
                           start=(ci == 0), stop=(ci == 3))
                    nc.scalar.activation(
                        lrT[:, ds(tg * 512, 512)], lps[:], AF.Identity,
                        bias=blr_sb[:, 0:1], scale=1.0,
                    )
                for i in range(NCH):
                    nc.gpsimd.dma_start(
                        out=lrn[:, i:i + 1], in_=lrT[0:1, ts(i, 128)]
                    )

            kcut = os.environ.get("KCUT", "")
            if kcut == "B":
                with tc.tile_pool(name="sbX", bufs=2) as SBX:
                    for tj in range(NCH):
                        zt = SBX.tile([128, C], f32, name="zt")
                        nc.vector.tensor_copy(zt[:], Kn[tj][:])
                        nc.sync.dma_start(out=out[ts(tj, 128), :], in_=zt[:])

            # ================= phase C: delta-rule recurrence ================
            # Pool scoping: construction pools (c2/c1/c1t) close after pair
            # NP-2 (all Mu/GX are built one pair ahead), freeing their PSUM
            # banks for the kv accumulators, which run during pair NP-1's
            # stalls.  PSG then hands its banks to psD for the output chain.
            if kcut in ("B",):
                pass
            elif True:
              with tc.tile_pool(name="sbC", bufs=4) as SBC, \
                 tc.tile_pool(name="sbR", bufs=4) as SBR, \
                 tc.tile_pool(name="sbD", bufs=4) as SBD, \
                 tc.tile_pool(name="psBIG", bufs=4, space="PSUM") as PSG:
                for ci in range(CT):
                    nc.gpsimd.memset(S_f32[ci][:], 0.0)

                def emit_V(i):
                    vps = PSG.tile([128, C], f32, name="big")
                    for ci in range(CT):
                        mm(vps[:], lhsT=xT[ci][:, ts(i, 128)],
                           rhs=WgT[ci][:, ds(2 * C, C)],
                           start=(ci == 0), stop=False)
                    mm(vps[:], lhsT=ones_bf[:], rhs=bgv_sb[:],
                       start=False, stop=True)
                    return vps

                def emit_P_S(i, close):
                    # P = Kraw_i S0 (+ cross term appended later for odd i)
                    pps = PSG.tile([128, C], f32, name="big")
                    for ci in range(CT):
                        mm(pps[:], lhsT=KTr[ci][:, ts(i, 128)], rhs=S_sb[ci][:],
                           start=(ci == 0), stop=(close and ci == 3))
                    return pps

                def emit_combine(i, vps, pps):
                    # rhs = rn * V  +  (-rn2) * P  (both legs on DVE so the
                    # chain has no cross-engine hop)
                    e1 = SBR.tile([128, C], bf16, name="e1")
                    nc.scalar.activation(
                        e1[:], vps[:], AF.Identity, scale=rn_all[:, i:i + 1]
                    )
                    if pps is None:
                        return e1
                    rhs = SBR.tile([128, C], bf16, name="rhs")
                    nc.vector.scalar_tensor_tensor(
                        rhs[:], pps[:], rn2n[:, i:i + 1], e1[:],
                        OP.mult, OP.add,
                    )
                    return rhs

                def emit_W(i, rhs):
                    wps = PSG.tile([128, C], f32, name="big")
                    mm(wps[:], lhsT=Mu[i][:], rhs=rhs[:], start=True, stop=True)
                    nc.vector.tensor_tensor(Wt[i][:], wps[:], rhs[:], OP.add)
                    nc.vector.tensor_scalar_mul(lru[i][:], Wt[i][:], lrn[:, i:i + 1])

                def emit_Supd(j):
                    a, b = 2 * j, 2 * j + 1
                    for ci in range(CT):
                        sd = PSG.tile([128, C], f32, name="big")
                        mm(sd[:], lhsT=Kn[a][:, ts(ci, 128)], rhs=Wt[a][:],
                           start=True, stop=False)
                        mm(sd[:], lhsT=Kn[b][:, ts(ci, 128)], rhs=Wt[b][:],
                           start=False, stop=True)
                        nc.vector.tensor_tensor(
                            S_f32[ci][:], sd[:], S_f32[ci][:], OP.add
                        )
                        nc.scalar.activation(S_sb[ci][:], S_f32[ci][:], AF.Identity)

                def do_pair(j, fill=None):
                    a, b = 2 * j, 2 * j + 1
                    vps_a = emit_V(a)
                    pps_a = emit_P_S(a, close=True) if j else None
                    vps_b = emit_V(b)
                    nBl = emit_gram(j + 1) if j < NP - 1 else None
                    if fill:
                        fill(0)
                    rhs_a = emit_combine(a, vps_a, pps_a)
                    emit_W(a, rhs_a)
                    pps_b = emit_P_S(b, close=False) if j else None
                    if j < NP - 1:
                        emit_constr(2 * j + 2, nBl[0])
                    if fill:
                        fill(1)
                    # cross term: P_b += gx^T W_a (closes / forms P_b group)
                    if pps_b is None:
                        pps_b = PSG.tile([128, C], f32, name="big")
                        mm(pps_b[:], lhsT=GX[j][:], rhs=Wt[a][:],
                           start=True, stop=True)
                    else:
                        mm(pps_b[:], lhsT=GX[j][:], rhs=Wt[a][:],
                           start=False, stop=True)
                    rhs_b = emit_combine(b, vps_b, pps_b)
                    emit_W(b, rhs_b)
                    if j < NP - 1:
                        emit_constr(2 * j + 3, nBl[1])
                    if fill:
                        fill(2)
                    if j < NP - 1:
                        emit_Supd(j)

                with tc.tile_pool(name="psC2", bufs=2, space="PSUM") as PSC2, \
                     tc.tile_pool(name="psC1", bufs=1, space="PSUM") as PSC1, \
                     tc.tile_pool(name="psCT", bufs=1, space="PSUM") as PSCT:

                    def emit_gram(j):
                        # pair grams: GA = [G_aa | gx], GB = G_bb; a=2j
                        a, b = 2 * j, 2 * j + 1
                        ga = PSC2.tile([128, 384], f32, name="c2")
                        for ci in range(CT):
                            mm(ga[:, 0:256], lhsT=KTr[ci][:, ts(a, 128)],
                               rhs=KTr[ci][:, ds(a * 128, 256)],
                               start=(ci == 0), stop=(ci == 3))
                        gb = PSC1.tile([128, 128], f32, name="c1")
                        for ci in range(CT):
                            mm(gb[:], lhsT=KTr[ci][:, ts(b, 128)],
                               rhs=KTr[ci][:, ts(b, 128)],
                               start=(ci == 0), stop=(ci == 3))
                        nc.scalar.activation(GX[j][:], ga[:, 128:256], AF.Identity)
                        # B = -tril(G,-1) * rn2 (rows): one fused DVE op each
                        Bl_a = SBC.tile([128, 128], bf16, name="Bla")
                        nc.vector.scalar_tensor_tensor(
                            Bl_a[:], ga[:, 0:128], rn2[:, a:a + 1], maskLn[:],
                            OP.mult, OP.mult,
                        )
                        Bl_b = SBC.tile([128, 128], bf16, name="Blb")
                        nc.vector.scalar_tensor_tensor(
                            Bl_b[:], gb[:], rn2[:, b:b + 1], maskLn[:],
                            OP.mult, OP.mult,
                        )
                        return Bl_a, Bl_b

                    def emit_constr(i, Bl):
                        # Mu[i] = (F1 + F2 + F1@F2)^T, F1 = B+B2+B3,
                        # F2 = B4+B8 (exact Neumann through degree 11).
                        # T1 = [Bu | B2u | B2l] lets [B3u | B4u] come from a
                        # single free-256 matmul with stationary B2l.
                        tps = PSCT.tile([128, 128], bf16, name="c1t")
                        nc.tensor.transpose(tps[:], Bl[:], ident_bf[:])
                        T1 = SBC.tile([128, 384], bf16, name="T1")
                        nc.scalar.activation(T1[:, 0:128], tps[:], AF.Identity)
                        Bu = T1[:, 0:128]

                        p1 = PSC2.tile([128, 384], f32, name="c2")
                        mm(p1[:, 0:128], lhsT=Bl[:], rhs=Bu, start=True, stop=True)
                        mm(p1[:, 128:256], lhsT=Bu, rhs=Bl[:], start=True, stop=True)
                        nc.scalar.activation(T1[:, 128:384], p1[:, 0:256], AF.Identity)
                        B2u, B2l = T1[:, 128:256], T1[:, 256:384]

                        pX = PSC2.tile([128, 384], f32, name="c2")
                        mm(pX[:, 0:256], lhsT=B2l, rhs=T1[:, 0:256],
                           start=True, stop=True)  # [B3u | B4u]
                        mm(pX[:, 256:384], lhsT=B2u, rhs=B2l, start=True, stop=True)
                        X = SBC.tile([128, 384], bf16, name="X")
                        nc.scalar.activation(X[:], pX[:], AF.Identity)
                        B3u, B4u, B4l = X[:, 0:128], X[:, 128:256], X[:, 256:384]

                        p3 = PSC2.tile([128, 384], f32, name="c2")
                        mm(p3[:, 0:128], lhsT=B4l, rhs=B4u, start=True, stop=True)
                        mm(p3[:, 128:256], lhsT=B4u, rhs=B4l, start=True, stop=True)
                        B8 = SBC.tile([128, 256], bf16, name="B8")
                        nc.scalar.activation(B8[:], p3[:, 0:256], AF.Identity)

                        # F1u = Bu + B2u + B3u; mp = (F1 F2)^T = F2u @ F1u;
                        # Mu = mp + F1u + F2u (adds done on DVE, not PE)
                        bb2 = SBC.tile([128, 128], bf16, name="bb2")
                        nc.gpsimd.tensor_tensor(bb2[:], Bu, B2u, OP.add)
                        F1u = SBC.tile([128, 128], bf16, name="F1u")
                        nc.gpsimd.tensor_tensor(F1u[:], B3u, bb2[:], OP.add)

                        F2 = SBC.tile([128, 256], bf16, name="F2")
                        nc.gpsimd.tensor_tensor(F2[:], B8[:], X[:, 128:384], OP.add)
                        F2u, F2l = F2[:, 0:128], F2[:, 128:256]
                        fsum = SBC.tile([128, 128], bf16, name="fsum")
                        nc.gpsimd.tensor_tensor(fsum[:], F1u[:], F2u, OP.add)

                        mp = PSC1.tile([128, 128], f32, name="c1")
                        mm(mp[:], lhsT=F2l, rhs=F1u[:], start=True, stop=True)
                        nc.vector.tensor_tensor(Mu[i][:], mp[:], fsum[:], OP.add)

                    # prologue: pair-0 construction
                    Bl_a, Bl_b = emit_gram(0)
                    emit_constr(0, Bl_a)
                    emit_constr(1, Bl_b)
                    for j in range(NP - 1):
                        do_pair(j)

                if kcut == "C":
                    do_pair(NP - 1)
                    with tc.tile_pool(name="sbX", bufs=2) as SBX:
                        for tj in range(NCH):
                            zt = SBX.tile([128, C], f32, name="zt")
                            nc.vector.tensor_copy(zt[:], Wt[tj][:])
                            nc.sync.dma_start(out=out[ts(tj, 128), :], in_=zt[:])
                else:
                    # ============== phase D: outputs =========================
                    with tc.tile_pool(name="psKV", bufs=1, space="PSUM") as PSKV:
                        kvt_ps = [PSKV.tile([128, C], f32, name=f"kvt{vi}")
                                  for vi in range(CT)]

                        def emit_kv(i0, i1):
                            for i in range(i0, i1):
                                for vi in range(CT):
                                    mm(kvt_ps[vi][:],
                                       lhsT=lru[i][:, ts(vi, 128)],
                                       rhs=Kn[i][:],
                                       start=(i == 0), stop=(i == NCH - 1))

                        emit_kv(0, 11)
                        do_pair(NP - 1, fill=lambda s: emit_kv(*[(11, 12), (12, 13),
                                                                 (13, 14)][s]))
                        emit_kv(14, NCH)
                        for vi in range(CT):
                            nc.any.tensor_copy(KVT[vi][:], kvt_ps[vi][:])

                    with tc.tile_pool(name="psD", bufs=2, space="PSUM") as PSD, \
                         tc.tile_pool(name="psDq", bufs=1, space="PSUM") as PSDQ:
                        for ki in range(CT):
                            kwp = PSD.tile([128, C], f32, name="dps")
                            for vi in range(CT):
                                mm(kwp[:], lhsT=KVT[vi][:, ts(ki, 128)],
                                   rhs=WoT[vi][:],
                                   start=(vi == 0), stop=(vi == 3))
                            nc.any.tensor_copy(KVW[ki][:], kwp[:])

                        # fold Q into the output: R = Wgq^T @ KVW,
                        # brow = bgq @ KVW + bo, out = x @ R + brow.
                        bqp = PSDQ.tile([1, C], f32, name="bqp")
                        for ri in range(CT):
                            mm(bqp[:], lhsT=bgq_sb[:, ri:ri + 1], rhs=KVW[ri][:],
                               start=(ri == 0), stop=(ri == 3))
                        brow = SBD.tile([1, C], bf16, name="brow")
                        nc.vector.tensor_tensor(brow[:], bqp[:], bo_sb[:], OP.add)
                        for ci in range(CT):
                            rps = PSD.tile([128, C], f32, name="dps")
                            for ri in range(CT):
                                mm(rps[:], lhsT=Wq[ri][:, ts(ci, 128)],
                                   rhs=KVW[ri][:],
                                   start=(ri == 0), stop=(ri == 3))
                            nc.any.tensor_copy(Rt[ci][:], rps[:])
                        b2p = PSD.tile([128, C], f32, name="dps")
                        mm(b2p[:], lhsT=ones_bf[:], rhs=brow[:], start=True, stop=True)
                        nc.any.tensor_copy(bo2_b[:], b2p[:])

                        for tj in range(NCH):
                            zps = PSD.tile([128, C], f32, name="dps")
                            for ci in range(CT):
                                mm(zps[:], lhsT=xT[ci][:, ts(tj, 128)],
                                   rhs=Rt[ci][:],
                                   start=(ci == 0), stop=(ci == 3))
                            zt = SBD.tile([128, C], f32, name="zt")
                            nc.vector.tensor_tensor(zt[:], zps[:], bo2_b[:], OP.add)
                            nc.sync.dma_start(out=out[ts(tj, 128), :], in_=zt[:])

                    if dbg:
                        for i in range(NCH):
                            dw = SBD.tile([128, C], f32, name="du")
                            nc.vector.tensor_copy(dw[:], Wt[i][:])
                            nc.sync.dma_start(out=dbg_W[ts(i, 128), :], in_=dw[:])
                            dk = SBD.tile([128, C], f32, name="du")
                            nc.vector.tensor_copy(dk[:], Kn[i][:])
                            nc.sync.dma_start(out=dbg_K[ts(i, 128), :], in_=dk[:])
                        for ci in range(CT):
                            dsb = SBD.tile([128, C], f32, name="du")
                            nc.vector.tensor_copy(dsb[:], S_f32[ci][:])
                            nc.sync.dma_start(out=dbg_S[ts(ci, 128), :], in_=dsb[:])
                            dkv = SBD.tile([128, C], f32, name="du")
                            nc.vector.tensor_copy(dkv[:], KVT[ci][:])
                            nc.sync.dma_start(out=dbg_KVT[ts(ci, 128), :], in_=dkv[:])

    nc.finalize()
    return nc


def _get_nc():
    if "nc" not in _CACHE:
        _CACHE["nc"] = _build()
    return _CACHE["nc"]


def _in_maps(inputs):
    def f(a):
        return np.ascontiguousarray(np.asarray(a, dtype=np.float32))

    x = f(inputs["x"])
    shared = {k: f(inputs[k]) for k in ("Wg", "bg", "Wlr", "blr", "Wo", "bo")}
    return [{"x": x[i], **shared} for i in range(N)]


def _run(in_maps, **kw):
    from concourse.bass_utils import run_bass_kernel_spmd

    nc = _get_nc()
    return run_bass_kernel_spmd(nc, in_maps, list(range(N)), **kw)


def kernel(**inputs) -> np.ndarray:
    res = _run(_in_maps(inputs))
    return np.stack([res.results[i]["out"] for i in range(N)]).astype(np.float32)


# revision 41
# speedup vs baseline: 1.0117x; 1.0117x over previous
# Bass/Trainium2 kernel for nn_Delta (DeltaNet-style recurrence).
#
# Problem (hardcoded): N=8, T=2048, C=512, fp32 I/O.
#   g = x @ Wg.T + bg ; q,k,v = split(g) ; lr = x @ Wlr.T + blr
#   khat = k / ||k||
#   delta-rule scan:  u_t = v_t - khat_t @ S ; S += outer(khat_t, u_t)
#   kv = sum_t khat_t (x) (lr_t * u_t) ; y = q @ kv ; out = y @ Wo.T + bo
#
# Sharding: data-parallel over N across the 8 cores (sample i -> core i),
# weights replicated. No collectives.
#
# Per-core algorithm: chunked parallel delta rule, chunk L=128, in
# "W-space": with D = diag(1/||k_raw||) per chunk and W = D^-1-free
# substitution W = D u, the recurrence becomes
#   (I + D^2 G) W = D V - D^2 (Kraw S + cross),   G = tril(Kraw Kraw^T, -1)
#   S += Kraw^T W,   kv = Kraw^T (lr . W)
# so only RAW projections appear in matmuls; the 1/||k|| and 1/||k||^2
# row scalings ride on PSUM evacuations (per-partition scale APs).
#
# (I + B)^-1 with B = -D^2 G is truncated exactly through degree 11 via
# the two-factor form (I+F1)(I+F2), F1 = B+B^2+B^3, F2 = B^4+B^8,
# applied merged: W = rhs + M' rhs with M' = F1+F2+F1F2  (measured
# truncation error 2.3e-3 in f64 on this data, far below the bf16 noise).
#
# Q is never materialized: y = q @ kv folds into out = x @ R + brow with
# R = Wgq^T (kv Wo^T) and brow = bgq (kv Wo^T) + bo, reusing resident xT.
#
# No DMA transposes: x, Wg, Wo are cast-DMA'd naturally and transposed
# on the tensor engine (transpose mode), as is Bl -> Bu.

import os

import numpy as np

N, T, C = 8, 2048, 512
L = 128
NCH = T // L  # 16 chunks
NP = NCH // 2  # 8 chunk pairs
CT = C // 128  # 4 c-tiles

_CACHE = {}


def _build():
    import concourse.bacc as bacc
    import concourse.mybir as mybir
    import concourse.tile as tile
    from concourse.bass import ts, ds
    from concourse.masks import make_identity, make_lower_triangular

    f32 = mybir.dt.float32
    bf16 = mybir.dt.bfloat16
    AF = mybir.ActivationFunctionType
    OP = mybir.AluOpType

    nc = bacc.Bacc("TRN2")
    x = nc.declare_dram_parameter("x", [T, C], f32, isOutput=False)
    Wg = nc.declare_dram_parameter("Wg", [3 * C, C], f32, isOutput=False)
    bg = nc.declare_dram_parameter("bg", [3 * C], f32, isOutput=False)
    Wlr = nc.declare_dram_parameter("Wlr", [1, C], f32, isOutput=False)
    blr = nc.declare_dram_parameter("blr", [1], f32, isOutput=False)
    Wo = nc.declare_dram_parameter("Wo", [C, C], f32, isOutput=False)
    bo = nc.declare_dram_parameter("bo", [C], f32, isOutput=False)
    out = nc.declare_dram_parameter("out", [T, C], f32, isOutput=True)
    dbg = os.environ.get("KDBG") == "1"
    if dbg:
        dbg_W = nc.declare_dram_parameter("dbg_W", [T, C], f32, isOutput=True)
        dbg_K = nc.declare_dram_parameter("dbg_K", [T, C], f32, isOutput=True)
        dbg_S = nc.declare_dram_parameter("dbg_S", [C, C], f32, isOutput=True)
        dbg_KVT = nc.declare_dram_parameter("dbg_KVT", [C, C], f32, isOutput=True)

    mm = nc.tensor.matmul

    with tile.TileContext(nc) as tc:
        with tc.tile_pool(name="persist", bufs=1) as P:
            # ---- constants / small tensors ----
            maskLn = P.tile([128, 128], f32, name="maskLn")
            make_lower_triangular(nc, maskLn[:], val=-1.0, diag=False)
            ones_bf = P.tile([1, 128], bf16, name="ones_bf")
            nc.vector.memset(ones_bf[:], 1.0)
            ident_bf = P.tile([128, 128], bf16, name="ident_bf")
            make_identity(nc, ident_bf[:])

            bgk_sb = P.tile([1, C], bf16, name="bgk_sb")
            bgv_sb = P.tile([1, C], bf16, name="bgv_sb")
            bo_sb = P.tile([1, C], bf16, name="bo_sb")
            bgq_sb = P.tile([128, CT], bf16, name="bgq_sb")
            WlrT_sb = P.tile([128, CT], bf16, name="WlrT_sb")
            blr_sb = P.tile([1, 1], f32, name="blr_sb")

            def load_small():
                nc.gpsimd.dma_start(out=bgk_sb[:], in_=bg[C:2 * C])
                nc.gpsimd.dma_start(out=bgv_sb[:], in_=bg[2 * C:3 * C])
                nc.gpsimd.dma_start(out=bo_sb[:], in_=bo[:])
                nc.gpsimd.dma_start(
                    out=bgq_sb[:], in_=bg[0:C].rearrange("(i p) -> p i", p=128)
                )
                nc.gpsimd.dma_start(
                    out=WlrT_sb[:], in_=Wlr[0, :].rearrange("(i p) -> p i", p=128)
                )
                nc.gpsimd.dma_start(out=blr_sb[:], in_=blr[:])

            # ---- persistent tensors ----
            xT = [P.tile([128, T], bf16, name=f"xT{i}") for i in range(CT)]
            WgT = [P.tile([128, 3 * C], bf16, name=f"WgT{i}") for i in range(CT)]
            WoT = [P.tile([128, C], bf16, name=f"WoT{i}") for i in range(CT)]
            KTr = [P.tile([128, T], bf16, name=f"KTr{i}") for i in range(CT)]
            Kn = [P.tile([128, C], bf16, name=f"Kn{i}") for i in range(NCH)]
            Wt = [P.tile([128, C], bf16, name=f"Wt{i}") for i in range(NCH)]
            lru = [P.tile([128, C], bf16, name=f"lru{i}") for i in range(NCH)]
            Wq = [P.tile([128, C], bf16, name=f"Wq{i}") for i in range(CT)]
            Rt = [P.tile([128, C], bf16, name=f"Rt{i}") for i in range(CT)]
            bo2_b = P.tile([128, C], f32, name="bo2_b")
            S_f32 = [P.tile([128, C], f32, name=f"Sf{i}") for i in range(CT)]
            S_sb = [P.tile([128, C], bf16, name=f"S{i}") for i in range(CT)]
            KVT = [P.tile([128, C], bf16, name=f"KVT{i}") for i in range(CT)]
            KVW = [P.tile([128, C], bf16, name=f"KVW{i}") for i in range(CT)]
            bgk_b = P.tile([128, C], bf16, name="bgk_b")
            bo_b = P.tile([128, C], f32, name="bo_b")
            lrT = P.tile([1, T], f32, name="lrT")
            lrn = P.tile([128, NCH], f32, name="lrn")
            n2_all = P.tile([128, NCH], f32, name="n2_all")
            rn2 = P.tile([128, NCH], f32, name="rn2")  # 1/||k||^2
            rn2n = P.tile([128, NCH], f32, name="rn2n")  # -1/||k||^2
            rn_all = P.tile([128, NCH], f32, name="rn_all")  # 1/||k||
            # per-chunk construction outputs (consumed next pair at latest)
            Mu = [P.tile([128, 128], bf16, name=f"Mu{i}") for i in range(NCH)]
            GX = [P.tile([128, 128], bf16, name=f"GX{i}") for i in range(NP)]

            # ============ phase A+B: loads, PE transposes, projections =======
            # Load order: x, Wg-k block, then the kps/KT chain runs on PE
            # while Wg-v, Wg-q, Wo stream in behind it.  Wg-q row tiles are
            # also kept in natural layout (Wq) for the output-side fold
            # y = x @ (Wgq^T kv Wo^T): Q is never materialized.
            with tc.tile_pool(name="stg", bufs=2) as STG, \
                 tc.tile_pool(name="sbB", bufs=4) as SBB, \
                 tc.tile_pool(name="psT", bufs=2, space="PSUM") as PST, \
                 tc.tile_pool(name="psB", bufs=2, space="PSUM") as PSB, \
                 tc.tile_pool(name="psKT", bufs=2, space="PSUM") as PSKT, \
                 tc.tile_pool(name="psL", bufs=1, space="PSUM") as PSL:

                def load_dma(src, row0):
                    grp = []
                    for jj in range(4):
                        t = STG.tile([128, C], bf16, name=f"stg{jj}")
                        nc.gpsimd.dma_start(
                            out=t[:],
                            in_=src[row0 + jj * 128:row0 + (jj + 1) * 128, :],
                        )
                        grp.append(t)
                    return grp

                def load_tp(grp, dstT, col0):
                    for ci in range(CT):
                        ps = PST.tile([128, 512], bf16, name="pst")
                        for jj in range(4):
                            nc.tensor.transpose(
                                ps[:, ts(jj, 128)],
                                grp[jj][:, ts(ci, 128)],
                                ident_bf[:],
                            )
                        nc.any.tensor_copy(dstT[ci][:, ds(col0, 512)], ps[:])

                def load_group(src, row0, dstT, col0):
                    load_tp(load_dma(src, row0), dstT, col0)

                load_group(x, 0, xT, 0)
                load_group(Wg, C, WgT, C)  # k rows
                load_small()
                load_group(x, 512, xT, 512)
                # bias broadcast rows -> [128, C] tiles (one matmul each)
                bps = PSL.tile([128, C], f32, name="bps")
                mm(bps[:], lhsT=ones_bf[:], rhs=bgk_sb[:], start=True, stop=True)
                nc.any.tensor_copy(bgk_b[:], bps[:])
                bps2 = PSL.tile([128, C], f32, name="bps")
                mm(bps2[:], lhsT=ones_bf[:], rhs=bo_sb[:], start=True, stop=True)
                nc.any.tensor_copy(bo_b[:], bps2[:])

                for tj in range(NCH):
                    if tj == 4:
                        load_group(x, 2 * 512, xT, 2 * 512)
                    elif tj == 6:
                        load_group(x, 3 * 512, xT, 3 * 512)
                    kps = PSB.tile([128, C], f32, name="kps")
                    for ci in range(CT):
                        mm(kps[:], lhsT=xT[ci][:, ts(tj, 128)],
                           rhs=WgT[ci][:, ds(C, C)],
                           start=(ci == 0), stop=(ci == 3))
                    # Kn = kps + bgk (broadcast tile); n2 = sum Kn^2
                    nc.vector.tensor_tensor(
                        Kn[tj][:], kps[:], bgk_b[:], OP.add
                    )
                    junk = SBB.tile([128, C], f32, name="junk")
                    nc.vector.scalar_tensor_tensor(
                        junk[:], Kn[tj][:], 1.0, Kn[tj][:], OP.mult, OP.mult,
                        accum_out=n2_all[:, tj:tj + 1],
                    )
                    if tj % 4 == 3:
                        for ci in range(CT):
                            ps = PSKT.tile([128, 512], bf16, name="pskt")
                            for jj in range(4):
                                nc.tensor.transpose(
                                    ps[:, ts(jj, 128)],
                                    Kn[tj - 3 + jj][:, ts(ci, 128)],
                                    ident_bf[:],
                                )
                            nc.any.tensor_copy(
                                KTr[ci][:, ds((tj - 3) * 128, 512)], ps[:]
                            )
                    if tj == 3:
                        load_group(Wg, 2 * C, WgT, 2 * C)  # v rows
                    elif tj == 7:
                        for jj in range(4):  # q rows, natural layout only
                            nc.gpsimd.dma_start(
                                out=Wq[jj][:], in_=Wg[jj * 128:(jj + 1) * 128, :]
                            )
                    elif tj == 11:
                        load_group(Wo, 0, WoT, 0)

                # row scalings
                nc.vector.reciprocal(rn2[:], n2_all[:])
                nc.vector.tensor_scalar_mul(rn2n[:], rn2[:], -1.0)
                nc.scalar.activation(rn_all[:], rn2[:], AF.Sqrt)

                # lr row: lrT[1, T] then scatter to lrn [128, NCH]
                for tg in range(4):
                    lps = PSL.tile([1, 512], f32, name="lps")
                    for ci in range(CT):
                        mm(lps[:], lhsT=WlrT_sb[:, ci:ci + 1],
                           rhs=xT[ci][:, ds(tg * 512, 512)],
                           start=(ci == 0), stop=(ci == 3))
                    nc.scalar.activation(
                        lrT[:, ds(tg * 512, 512)], lps[:], AF.Identity,
                        bias=blr_sb[:, 0:1], scale=1.0,
                    )
                for i in range(NCH):
                    nc.gpsimd.dma_start(
                        out=lrn[:, i:i + 1], in_=lrT[0:1, ts(i, 128)]
                    )

            kcut = os.environ.get("KCUT", "")
            if kcut == "B":
                with tc.tile_pool(name="sbX", bufs=2) as SBX:
                    for tj in range(NCH):
                        zt = SBX.tile([128, C], f32, name="zt")
                        nc.vector.tensor_copy(zt[:], Kn[tj][:])
                        nc.sync.dma_start(out=out[ts(tj, 128), :], in_=zt[:])

            # ================= phase C: delta-rule recurrence ================
            # Pool scoping: construction pools (c2/c1/c1t) close after pair
            # NP-2 (all Mu/GX are built one pair ahead), freeing their PSUM
            # banks for the kv accumulators, which run during pair NP-1's
            # stalls.  PSG then hands its banks to psD for the output chain.
            if kcut in ("B",):
                pass
            elif True:
              with tc.tile_pool(name="sbC", bufs=4) as SBC, \
                 tc.tile_pool(name="sbR", bufs=4) as SBR, \
                 tc.tile_pool(name="sbD", bufs=4) as SBD, \
                 tc.tile_pool(name="psBIG", bufs=4, space="PSUM") as PSG:
                for ci in range(CT):
                    nc.gpsimd.memset(S_f32[ci][:], 0.0)

                def emit_V(i):
                    vps = PSG.tile([128, C], f32, name="big")
                    for ci in range(CT):
                        mm(vps[:], lhsT=xT[ci][:, ts(i, 128)],
                           rhs=WgT[ci][:, ds(2 * C, C)],
                           start=(ci == 0), stop=False)
                    mm(vps[:], lhsT=ones_bf[:], rhs=bgv_sb[:],
                       start=False, stop=True)
                    return vps

                def emit_P_S(i, close):
                    # P = Kraw_i S0 (+ cross term appended later for odd i)
                    pps = PSG.tile([128, C], f32, name="big")
                    for ci in range(CT):
                        mm(pps[:], lhsT=KTr[ci][:, ts(i, 128)], rhs=S_sb[ci][:],
                           start=(ci == 0), stop=(close and ci == 3))
                    return pps

                def emit_combine(i, vps, pps):
                    # rhs = rn * V  +  (-rn2) * P  (both legs on DVE so the
                    # chain has no cross-engine hop)
                    e1 = SBR.tile([128, C], bf16, name="e1")
                    nc.scalar.activation(
                        e1[:], vps[:], AF.Identity, scale=rn_all[:, i:i + 1]
                    )
                    if pps is None:
                        return e1
                    rhs = SBR.tile([128, C], bf16, name="rhs")
                    nc.vector.scalar_tensor_tensor(
                        rhs[:], pps[:], rn2n[:, i:i + 1], e1[:],
                        OP.mult, OP.add,
                    )
                    return rhs

                def emit_W(i, rhs):
                    wps = PSG.tile([128, C], f32, name="big")
                    mm(wps[:], lhsT=Mu[i][:], rhs=rhs[:], start=True, stop=True)
                    nc.vector.tensor_tensor(Wt[i][:], wps[:], rhs[:], OP.add)
                    nc.vector.tensor_scalar_mul(lru[i][:], Wt[i][:], lrn[:, i:i + 1])

                def emit_Supd(j):
                    a, b = 2 * j, 2 * j + 1
                    for ci in range(CT):
                        sd = PSG.tile([128, C], f32, name="big")
                        mm(sd[:], lhsT=Kn[a][:, ts(ci, 128)], rhs=Wt[a][:],
                           start=True, stop=False)
                        mm(sd[:], lhsT=Kn[b][:, ts(ci, 128)], rhs=Wt[b][:],
                           start=False, stop=True)
                        nc.vector.tensor_tensor(
                            S_f32[ci][:], sd[:], S_f32[ci][:], OP.add
                        )
                        nc.scalar.activation(S_sb[ci][:], S_f32[ci][:], AF.Identity)

                def do_pair(j, fill=None):
                    a, b = 2 * j, 2 * j + 1
                    vps_a = emit_V(a)
                    pps_a = emit_P_S(a, close=True) if j else None
                    vps_b = emit_V(b)
                    nBl = emit_gram(j + 1) if j < NP - 1 else None
                    if fill:
                        fill(0)
                    rhs_a = emit_combine(a, vps_a, pps_a)
                    emit_W(a, rhs_a)
                    pps_b = emit_P_S(b, close=False) if j else None
                    fin_a = emit_constr(2 * j + 2, nBl[0]) if j < NP - 1 else None
                    if fill:
                        fill(1)
                    # cross term: P_b += gx^T W_a (closes / forms P_b group)
                    if pps_b is None:
                        pps_b = PSG.tile([128, C], f32, name="big")
                        mm(pps_b[:], lhsT=GX[j][:], rhs=Wt[a][:],
                           start=True, stop=True)
                    else:
                        mm(pps_b[:], lhsT=GX[j][:], rhs=Wt[a][:],
                           start=False, stop=True)
                    rhs_b = emit_combine(b, vps_b, pps_b)
                    emit_W(b, rhs_b)
                    if fin_a is not None:
                        fin_a()
                    fin_b = emit_constr(2 * j + 3, nBl[1]) if j < NP - 1 else None
                    if fill:
                        fill(2)
                    if j < NP - 1:
                        emit_Supd(j)
                    if fin_b is not None:
                        fin_b()

                with tc.tile_pool(name="psC2", bufs=2, space="PSUM") as PSC2, \
                     tc.tile_pool(name="psC1", bufs=1, space="PSUM") as PSC1, \
                     tc.tile_pool(name="psCT", bufs=1, space="PSUM") as PSCT:

                    def emit_gram(j):
                        # pair grams: GA = [G_aa | gx], GB = G_bb; a=2j
                        a, b = 2 * j, 2 * j + 1
                        ga = PSC2.tile([128, 384], f32, name="c2")
                        for ci in range(CT):
                            mm(ga[:, 0:256], lhsT=KTr[ci][:, ts(a, 128)],
                               rhs=KTr[ci][:, ds(a * 128, 256)],
                               start=(ci == 0), stop=(ci == 3))
                        gb = PSC1.tile([128, 128], f32, name="c1")
                        for ci in range(CT):
                            mm(gb[:], lhsT=KTr[ci][:, ts(b, 128)],
                               rhs=KTr[ci][:, ts(b, 128)],
                               start=(ci == 0), stop=(ci == 3))
                        nc.scalar.activation(GX[j][:], ga[:, 128:256], AF.Identity)
                        # B = -tril(G,-1) * rn2 (rows): one fused DVE op each
                        Bl_a = SBC.tile([128, 128], bf16, name="Bla")
                        nc.vector.scalar_tensor_tensor(
                            Bl_a[:], ga[:, 0:128], rn2[:, a:a + 1], maskLn[:],
                            OP.mult, OP.mult,
                        )
                        Bl_b = SBC.tile([128, 128], bf16, name="Blb")
                        nc.vector.scalar_tensor_tensor(
                            Bl_b[:], gb[:], rn2[:, b:b + 1], maskLn[:],
                            OP.mult, OP.mult,
                        )
                        return Bl_a, Bl_b

                    def emit_constr(i, Bl):
                        # Mu[i] = (F1 + F2 + F1@F2)^T, F1 = B+B2+B3,
                        # F2 = B4+B8 (exact Neumann through degree 11).
                        # T1 = [Bu | B2u | B2l] lets [B3u | B4u] come from a
                        # single free-256 matmul with stationary B2l.
                        tps = PSCT.tile([128, 128], bf16, name="c1t")
                        nc.tensor.transpose(tps[:], Bl[:], ident_bf[:])
                        T1 = SBC.tile([128, 384], bf16, name="T1")
                        nc.scalar.activation(T1[:, 0:128], tps[:], AF.Identity)
                        Bu = T1[:, 0:128]

                        p1 = PSC2.tile([128, 384], f32, name="c2")
                        mm(p1[:, 0:128], lhsT=Bl[:], rhs=Bu, start=True, stop=True)
                        mm(p1[:, 128:256], lhsT=Bu, rhs=Bl[:], start=True, stop=True)
                        nc.scalar.activation(T1[:, 128:384], p1[:, 0:256], AF.Identity)
                        B2u, B2l = T1[:, 128:256], T1[:, 256:384]

                        pX = PSC2.tile([128, 384], f32, name="c2")
                        mm(pX[:, 0:256], lhsT=B2l, rhs=T1[:, 0:256],
                           start=True, stop=True)  # [B3u | B4u]
                        mm(pX[:, 256:384], lhsT=B2u, rhs=B2l, start=True, stop=True)
                        X = SBC.tile([128, 384], bf16, name="X")
                        nc.scalar.activation(X[:], pX[:], AF.Identity)
                        B3u, B4u, B4l = X[:, 0:128], X[:, 128:256], X[:, 256:384]

                        p3 = PSC2.tile([128, 384], f32, name="c2")
                        mm(p3[:, 0:128], lhsT=B4l, rhs=B4u, start=True, stop=True)
                        mm(p3[:, 128:256], lhsT=B4u, rhs=B4l, start=True, stop=True)
                        B8 = SBC.tile([128, 256], bf16, name="B8")
                        nc.scalar.activation(B8[:], p3[:, 0:256], AF.Identity)

                        # F1u = Bu + B2u + B3u; mp = (F1 F2)^T = F2u @ F1u;
                        # Mu = mp + F1u + F2u.  The DVE adds and the merge mm
                        # are DEFERRED (returned as a closure) so the pair
                        # loop can emit them after its chain-critical DVE ops
                        # -- keeps the in-order DVE queue clear for rhs/W.
                        def finish():
                            bb2 = SBC.tile([128, 128], bf16, name="bb2")
                            nc.vector.tensor_tensor(bb2[:], Bu, B2u, OP.add)
                            F1u = SBC.tile([128, 128], bf16, name="F1u")
                            nc.vector.tensor_tensor(F1u[:], B3u, bb2[:], OP.add)

                            F2 = SBC.tile([128, 256], bf16, name="F2")
                            nc.vector.tensor_tensor(
                                F2[:], B8[:], X[:, 128:384], OP.add
                            )
                            F2u, F2l = F2[:, 0:128], F2[:, 128:256]
                            fsum = SBC.tile([128, 128], bf16, name="fsum")
                            nc.vector.tensor_tensor(fsum[:], F1u[:], F2u, OP.add)

                            mp = PSC1.tile([128, 128], f32, name="c1")
                            mm(mp[:], lhsT=F2l, rhs=F1u[:], start=True, stop=True)
                            nc.vector.tensor_tensor(Mu[i][:], mp[:], fsum[:], OP.add)
                        return finish

                    # prologue: pair-0 construction
                    Bl_a, Bl_b = emit_gram(0)
                    emit_constr(0, Bl_a)()
                    emit_constr(1, Bl_b)()
                    for j in range(NP - 1):
                        do_pair(j)

                if kcut == "C":
                    do_pair(NP - 1)
                    with tc.tile_pool(name="sbX", bufs=2) as SBX:
                        for tj in range(NCH):
                            zt = SBX.tile([128, C], f32, name="zt")
                            nc.vector.tensor_copy(zt[:], Wt[tj][:])
                            nc.sync.dma_start(out=out[ts(tj, 128), :], in_=zt[:])
                else:
                    # ============== phase D: outputs =========================
                    with tc.tile_pool(name="psKV", bufs=1, space="PSUM") as PSKV:
                        kvt_ps = [PSKV.tile([128, C], f32, name=f"kvt{vi}")
                                  for vi in range(CT)]

                        def emit_kv(i0, i1):
                            for i in range(i0, i1):
                                for vi in range(CT):
                                    mm(kvt_ps[vi][:],
                                       lhsT=lru[i][:, ts(vi, 128)],
                                       rhs=Kn[i][:],
                                       start=(i == 0), stop=(i == NCH - 1))

                        emit_kv(0, 11)
                        do_pair(NP - 1, fill=lambda s: emit_kv(*[(11, 12), (12, 13),
                                                                 (13, 14)][s]))
                        emit_kv(14, NCH)
                        for vi in range(CT):
                            nc.any.tensor_copy(KVT[vi][:], kvt_ps[vi][:])

                    with tc.tile_pool(name="psD", bufs=2, space="PSUM") as PSD, \
                         tc.tile_pool(name="psDq", bufs=1, space="PSUM") as PSDQ:
                        for ki in range(CT):
                            kwp = PSD.tile([128, C], f32, name="dps")
                            for vi in range(CT):
                                mm(kwp[:], lhsT=KVT[vi][:, ts(ki, 128)],
                                   rhs=WoT[vi][:],
                                   start=(vi == 0), stop=(vi == 3))
                            nc.any.tensor_copy(KVW[ki][:], kwp[:])

                        # fold Q into the output: R = Wgq^T @ KVW,
                        # brow = bgq @ KVW + bo, out = x @ R + brow.
                        bqp = PSDQ.tile([1, C], f32, name="bqp")
                        for ri in range(CT):
                            mm(bqp[:], lhsT=bgq_sb[:, ri:ri + 1], rhs=KVW[ri][:],
                               start=(ri == 0), stop=(ri == 3))
                        brow = SBD.tile([1, C], bf16, name="brow")
                        nc.vector.tensor_tensor(brow[:], bqp[:], bo_sb[:], OP.add)
                        for ci in range(CT):
                            rps = PSD.tile([128, C], f32, name="dps")
                            for ri in range(CT):
                                mm(rps[:], lhsT=Wq[ri][:, ts(ci, 128)],
                                   rhs=KVW[ri][:],
                                   start=(ri == 0), stop=(ri == 3))
                            nc.any.tensor_copy(Rt[ci][:], rps[:])
                        b2p = PSD.tile([128, C], f32, name="dps")
                        mm(b2p[:], lhsT=ones_bf[:], rhs=brow[:], start=True, stop=True)
                        nc.any.tensor_copy(bo2_b[:], b2p[:])

                        for tj in range(NCH):
                            zps = PSD.tile([128, C], f32, name="dps")
                            for ci in range(CT):
                                mm(zps[:], lhsT=xT[ci][:, ts(tj, 128)],
                                   rhs=Rt[ci][:],
                                   start=(ci == 0), stop=(ci == 3))
                            zt = SBD.tile([128, C], f32, name="zt")
                            nc.vector.tensor_tensor(zt[:], zps[:], bo2_b[:], OP.add)
                            nc.sync.dma_start(out=out[ts(tj, 128), :], in_=zt[:])

                    if dbg:
                        for i in range(NCH):
                            dw = SBD.tile([128, C], f32, name="du")
                            nc.vector.tensor_copy(dw[:], Wt[i][:])
                            nc.sync.dma_start(out=dbg_W[ts(i, 128), :], in_=dw[:])
                            dk = SBD.tile([128, C], f32, name="du")
                            nc.vector.tensor_copy(dk[:], Kn[i][:])
                            nc.sync.dma_start(out=dbg_K[ts(i, 128), :], in_=dk[:])
                        for ci in range(CT):
                            dsb = SBD.tile([128, C], f32, name="du")
                            nc.vector.tensor_copy(dsb[:], S_f32[ci][:])
                            nc.sync.dma_start(out=dbg_S[ts(ci, 128), :], in_=dsb[:])
                            dkv = SBD.tile([128, C], f32, name="du")
                            nc.vector.tensor_copy(dkv[:], KVT[ci][:])
                            nc.sync.dma_start(out=dbg_KVT[ts(ci, 128), :], in_=dkv[:])

    nc.finalize()
    return nc


def _get_nc():
    if "nc" not in _CACHE:
        _CACHE["nc"] = _build()
    return _CACHE["nc"]


def _in_maps(inputs):
    def f(a):
        return np.ascontiguousarray(np.asarray(a, dtype=np.float32))

    x = f(inputs["x"])
    shared = {k: f(inputs[k]) for k in ("Wg", "bg", "Wlr", "blr", "Wo", "bo")}
    return [{"x": x[i], **shared} for i in range(N)]


def _run(in_maps, **kw):
    from concourse.bass_utils import run_bass_kernel_spmd

    nc = _get_nc()
    return run_bass_kernel_spmd(nc, in_maps, list(range(N)), **kw)


def kernel(**inputs) -> np.ndarray:
    res = _run(_in_maps(inputs))
    return np.stack([res.results[i]["out"] for i in range(N)]).astype(np.float32)


# revision 42
# speedup vs baseline: 1.0404x; 1.0284x over previous
# Bass/Trainium2 kernel for nn_Delta (DeltaNet-style recurrence).
#
# Problem (hardcoded): N=8, T=2048, C=512, fp32 I/O.
#   g = x @ Wg.T + bg ; q,k,v = split(g) ; lr = x @ Wlr.T + blr
#   khat = k / ||k||
#   delta-rule scan:  u_t = v_t - khat_t @ S ; S += outer(khat_t, u_t)
#   kv = sum_t khat_t (x) (lr_t * u_t) ; y = q @ kv ; out = y @ Wo.T + bo
#
# Sharding: data-parallel over N across the 8 cores (sample i -> core i),
# weights replicated. No collectives.
#
# Per-core algorithm: chunked parallel delta rule, chunk L=128, in
# "W-space": with D = diag(1/||k_raw||) per chunk and W = D^-1-free
# substitution W = D u, the recurrence becomes
#   (I + D^2 G) W = D V - D^2 (Kraw S + cross),   G = tril(Kraw Kraw^T, -1)
#   S += Kraw^T W,   kv = Kraw^T (lr . W)
# so only RAW projections appear in matmuls; the 1/||k|| and 1/||k||^2
# row scalings ride on PSUM evacuations (per-partition scale APs).
#
# (I + B)^-1 with B = -D^2 G is truncated exactly through degree 11 via
# the two-factor form (I+F1)(I+F2), F1 = B+B^2+B^3, F2 = B^4+B^8,
# applied merged: W = rhs + M' rhs with M' = F1+F2+F1F2  (measured
# truncation error 2.3e-3 in f64 on this data, far below the bf16 noise).
#
# Q is never materialized: y = q @ kv folds into out = x @ R + brow with
# R = Wgq^T (kv Wo^T) and brow = bgq (kv Wo^T) + bo, reusing resident xT.
#
# No DMA transposes: x, Wg, Wo are cast-DMA'd naturally and transposed
# on the tensor engine (transpose mode), as is Bl -> Bu.

import os

import numpy as np

N, T, C = 8, 2048, 512
L = 128
NCH = T // L  # 16 chunks
NP = NCH // 2  # 8 chunk pairs
CT = C // 128  # 4 c-tiles

_CACHE = {}


def _build():
    import concourse.bacc as bacc
    import concourse.mybir as mybir
    import concourse.tile as tile
    from concourse.bass import ts, ds
    from concourse.masks import make_identity, make_lower_triangular

    f32 = mybir.dt.float32
    bf16 = mybir.dt.bfloat16
    AF = mybir.ActivationFunctionType
    OP = mybir.AluOpType

    nc = bacc.Bacc("TRN2")
    x = nc.declare_dram_parameter("x", [T, C], f32, isOutput=False)
    Wg = nc.declare_dram_parameter("Wg", [3 * C, C], f32, isOutput=False)
    bg = nc.declare_dram_parameter("bg", [3 * C], f32, isOutput=False)
    Wlr = nc.declare_dram_parameter("Wlr", [1, C], f32, isOutput=False)
    blr = nc.declare_dram_parameter("blr", [1], f32, isOutput=False)
    Wo = nc.declare_dram_parameter("Wo", [C, C], f32, isOutput=False)
    bo = nc.declare_dram_parameter("bo", [C], f32, isOutput=False)
    out = nc.declare_dram_parameter("out", [T, C], f32, isOutput=True)
    dbg = os.environ.get("KDBG") == "1"
    if dbg:
        dbg_W = nc.declare_dram_parameter("dbg_W", [T, C], f32, isOutput=True)
        dbg_K = nc.declare_dram_parameter("dbg_K", [T, C], f32, isOutput=True)
        dbg_S = nc.declare_dram_parameter("dbg_S", [C, C], f32, isOutput=True)
        dbg_KVT = nc.declare_dram_parameter("dbg_KVT", [C, C], f32, isOutput=True)

    mm = nc.tensor.matmul

    with tile.TileContext(nc) as tc:
        with tc.tile_pool(name="persist", bufs=1) as P:
            # ---- constants / small tensors ----
            maskLn = P.tile([128, 128], f32, name="maskLn")
            make_lower_triangular(nc, maskLn[:], val=-1.0, diag=False)
            ones_bf = P.tile([1, 128], bf16, name="ones_bf")
            nc.vector.memset(ones_bf[:], 1.0)
            ident_bf = P.tile([128, 128], bf16, name="ident_bf")
            make_identity(nc, ident_bf[:])

            bgk_sb = P.tile([1, C], bf16, name="bgk_sb")
            bgv_sb = P.tile([1, C], bf16, name="bgv_sb")
            bo_sb = P.tile([1, C], bf16, name="bo_sb")
            bgq_sb = P.tile([128, CT], bf16, name="bgq_sb")
            WlrT_sb = P.tile([128, CT], bf16, name="WlrT_sb")
            blr_sb = P.tile([1, 1], f32, name="blr_sb")

            def load_small():
                nc.gpsimd.dma_start(out=bgk_sb[:], in_=bg[C:2 * C])
                nc.gpsimd.dma_start(out=bgv_sb[:], in_=bg[2 * C:3 * C])
                nc.gpsimd.dma_start(out=bo_sb[:], in_=bo[:])
                nc.gpsimd.dma_start(
                    out=bgq_sb[:], in_=bg[0:C].rearrange("(i p) -> p i", p=128)
                )
                nc.gpsimd.dma_start(
                    out=WlrT_sb[:], in_=Wlr[0, :].rearrange("(i p) -> p i", p=128)
                )
                nc.gpsimd.dma_start(out=blr_sb[:], in_=blr[:])

            # ---- persistent tensors ----
            xT = [P.tile([128, T], bf16, name=f"xT{i}") for i in range(CT)]
            WgT = [P.tile([128, 3 * C], bf16, name=f"WgT{i}") for i in range(CT)]
            WoT = [P.tile([128, C], bf16, name=f"WoT{i}") for i in range(CT)]
            KTr = [P.tile([128, T], bf16, name=f"KTr{i}") for i in range(CT)]
            Kn = [P.tile([128, C], bf16, name=f"Kn{i}") for i in range(NCH)]
            Wt = [P.tile([128, C], bf16, name=f"Wt{i}") for i in range(NCH)]
            lru = [P.tile([128, C], bf16, name=f"lru{i}") for i in range(NCH)]
            Wq = [P.tile([128, C], bf16, name=f"Wq{i}") for i in range(CT)]
            Rt = [P.tile([128, C], bf16, name=f"Rt{i}") for i in range(CT)]
            bo2_b = P.tile([128, C], f32, name="bo2_b")
            S_f32 = [P.tile([128, C], f32, name=f"Sf{i}") for i in range(CT)]
            S_sb = [P.tile([128, C], bf16, name=f"S{i}") for i in range(CT)]
            KVT = [P.tile([128, C], bf16, name=f"KVT{i}") for i in range(CT)]
            KVW = [P.tile([128, C], bf16, name=f"KVW{i}") for i in range(CT)]
            bgk_b = P.tile([128, C], bf16, name="bgk_b")
            bo_b = P.tile([128, C], f32, name="bo_b")
            lrT = P.tile([1, T], f32, name="lrT")
            lrn = P.tile([128, NCH], f32, name="lrn")
            n2_all = P.tile([128, NCH], f32, name="n2_all")
            rn2 = P.tile([128, NCH], f32, name="rn2")  # 1/||k||^2
            rn2n = P.tile([128, NCH], f32, name="rn2n")  # -1/||k||^2
            rn_all = P.tile([128, NCH], f32, name="rn_all")  # 1/||k||
            # per-chunk construction outputs (consumed next pair at latest)
            Mu = [P.tile([128, 128], bf16, name=f"Mu{i}") for i in range(NCH)]
            GX = [P.tile([128, 128], bf16, name=f"GX{i}") for i in range(NP)]

            # ============ phase A+B: loads, PE transposes, projections =======
            # Load order: x, Wg-k block, then the kps/KT chain runs on PE
            # while Wg-v, Wg-q, Wo stream in behind it.  Wg-q row tiles are
            # also kept in natural layout (Wq) for the output-side fold
            # y = x @ (Wgq^T kv Wo^T): Q is never materialized.
            with tc.tile_pool(name="stg", bufs=2) as STG, \
                 tc.tile_pool(name="sbB", bufs=4) as SBB, \
                 tc.tile_pool(name="psT", bufs=2, space="PSUM") as PST, \
                 tc.tile_pool(name="psB", bufs=2, space="PSUM") as PSB, \
                 tc.tile_pool(name="psKT", bufs=2, space="PSUM") as PSKT, \
                 tc.tile_pool(name="psL", bufs=1, space="PSUM") as PSL:

                def load_dma(src, row0):
                    grp = []
                    for jj in range(4):
                        t = STG.tile([128, C], bf16, name=f"stg{jj}")
                        nc.gpsimd.dma_start(
                            out=t[:],
                            in_=src[row0 + jj * 128:row0 + (jj + 1) * 128, :],
                        )
                        grp.append(t)
                    return grp

                def load_tp(grp, dstT, col0):
                    for ci in range(CT):
                        ps = PST.tile([128, 512], bf16, name="pst")
                        for jj in range(4):
                            nc.tensor.transpose(
                                ps[:, ts(jj, 128)],
                                grp[jj][:, ts(ci, 128)],
                                ident_bf[:],
                            )
                        nc.any.tensor_copy(dstT[ci][:, ds(col0, 512)], ps[:])

                def load_group(src, row0, dstT, col0):
                    load_tp(load_dma(src, row0), dstT, col0)

                load_group(x, 0, xT, 0)
                load_group(Wg, C, WgT, C)  # k rows
                load_small()
                load_group(x, 512, xT, 512)
                # bias broadcast rows -> [128, C] tiles (one matmul each)
                bps = PSL.tile([128, C], f32, name="bps")
                mm(bps[:], lhsT=ones_bf[:], rhs=bgk_sb[:], start=True, stop=True)
                nc.any.tensor_copy(bgk_b[:], bps[:])
                bps2 = PSL.tile([128, C], f32, name="bps")
                mm(bps2[:], lhsT=ones_bf[:], rhs=bo_sb[:], start=True, stop=True)
                nc.any.tensor_copy(bo_b[:], bps2[:])

                for tj in range(NCH):
                    if tj == 4:
                        load_group(x, 2 * 512, xT, 2 * 512)
                    elif tj == 6:
                        load_group(x, 3 * 512, xT, 3 * 512)
                    kps = PSB.tile([128, C], f32, name="kps")
                    for ci in range(CT):
                        mm(kps[:], lhsT=xT[ci][:, ts(tj, 128)],
                           rhs=WgT[ci][:, ds(C, C)],
                           start=(ci == 0), stop=(ci == 3))
                    # Kn = kps + bgk (broadcast tile); n2 = sum Kn^2
                    nc.vector.tensor_tensor(
                        Kn[tj][:], kps[:], bgk_b[:], OP.add
                    )
                    junk = SBB.tile([128, C], f32, name="junk")
                    nc.vector.scalar_tensor_tensor(
                        junk[:], Kn[tj][:], 1.0, Kn[tj][:], OP.mult, OP.mult,
                        accum_out=n2_all[:, tj:tj + 1],
                    )
                    if tj % 4 == 3:
                        for ci in range(CT):
                            ps = PSKT.tile([128, 512], bf16, name="pskt")
                            for jj in range(4):
                                nc.tensor.transpose(
                                    ps[:, ts(jj, 128)],
                                    Kn[tj - 3 + jj][:, ts(ci, 128)],
                                    ident_bf[:],
                                )
                            nc.any.tensor_copy(
                                KTr[ci][:, ds((tj - 3) * 128, 512)], ps[:]
                            )
                    if tj == 3:
                        load_group(Wg, 2 * C, WgT, 2 * C)  # v rows
                    elif tj == 7:
                        for jj in range(4):  # q rows, natural layout only
                            nc.gpsimd.dma_start(
                                out=Wq[jj][:], in_=Wg[jj * 128:(jj + 1) * 128, :]
                            )
                    elif tj == 11:
                        load_group(Wo, 0, WoT, 0)

                # row scalings
                nc.vector.reciprocal(rn2[:], n2_all[:])
                nc.vector.tensor_scalar_mul(rn2n[:], rn2[:], -1.0)
                nc.scalar.activation(rn_all[:], rn2[:], AF.Sqrt)

                # lr row: lrT[1, T] then scatter to lrn [128, NCH]
                for tg in range(4):
                    lps = PSL.tile([1, 512], f32, name="lps")
                    for ci in range(CT):
                        mm(lps[:], lhsT=WlrT_sb[:, ci:ci + 1],
                           rhs=xT[ci][:, ds(tg * 512, 512)],
                           start=(ci == 0), stop=(ci == 3))
                    nc.scalar.activation(
                        lrT[:, ds(tg * 512, 512)], lps[:], AF.Identity,
                        bias=blr_sb[:, 0:1], scale=1.0,
                    )
                for i in range(NCH):
                    nc.gpsimd.dma_start(
                        out=lrn[:, i:i + 1], in_=lrT[0:1, ts(i, 128)]
                    )

            kcut = os.environ.get("KCUT", "")
            if kcut == "B":
                with tc.tile_pool(name="sbX", bufs=2) as SBX:
                    for tj in range(NCH):
                        zt = SBX.tile([128, C], f32, name="zt")
                        nc.vector.tensor_copy(zt[:], Kn[tj][:])
                        nc.sync.dma_start(out=out[ts(tj, 128), :], in_=zt[:])

            # ================= phase C: delta-rule recurrence ================
            # Pool scoping: construction pools (c2/c1/c1t) close after pair
            # NP-2 (all Mu/GX are built one pair ahead), freeing their PSUM
            # banks for the kv accumulators, which run during pair NP-1's
            # stalls.  PSG then hands its banks to psD for the output chain.
            if kcut in ("B",):
                pass
            elif True:
              with tc.tile_pool(name="sbC", bufs=4) as SBC, \
                 tc.tile_pool(name="sbR", bufs=4) as SBR, \
                 tc.tile_pool(name="sbD", bufs=4) as SBD, \
                 tc.tile_pool(name="psBIG", bufs=4, space="PSUM") as PSG:
                for ci in range(CT):
                    nc.gpsimd.memset(S_f32[ci][:], 0.0)

                def emit_V(i):
                    vps = PSG.tile([128, C], f32, name="big")
                    for ci in range(CT):
                        mm(vps[:], lhsT=xT[ci][:, ts(i, 128)],
                           rhs=WgT[ci][:, ds(2 * C, C)],
                           start=(ci == 0), stop=False)
                    mm(vps[:], lhsT=ones_bf[:], rhs=bgv_sb[:],
                       start=False, stop=True)
                    return vps

                def emit_P_S(i, close):
                    # P = Kraw_i S0 (+ cross term appended later for odd i)
                    pps = PSG.tile([128, C], f32, name="big")
                    for ci in range(CT):
                        mm(pps[:], lhsT=KTr[ci][:, ts(i, 128)], rhs=S_sb[ci][:],
                           start=(ci == 0), stop=(close and ci == 3))
                    return pps

                def emit_combine(i, vps, pps):
                    # rhs = rn * V  +  (-rn2) * P  (both legs on DVE so the
                    # chain has no cross-engine hop)
                    e1 = SBR.tile([128, C], bf16, name="e1")
                    nc.scalar.activation(
                        e1[:], vps[:], AF.Identity, scale=rn_all[:, i:i + 1]
                    )
                    if pps is None:
                        return e1
                    rhs = SBR.tile([128, C], bf16, name="rhs")
                    nc.vector.scalar_tensor_tensor(
                        rhs[:], pps[:], rn2n[:, i:i + 1], e1[:],
                        OP.mult, OP.add,
                    )
                    return rhs

                def emit_W(i, rhs):
                    wps = PSG.tile([128, C], f32, name="big")
                    mm(wps[:], lhsT=Mu[i][:], rhs=rhs[:], start=True, stop=True)
                    nc.vector.tensor_tensor(Wt[i][:], wps[:], rhs[:], OP.add)
                    nc.vector.tensor_scalar_mul(lru[i][:], Wt[i][:], lrn[:, i:i + 1])

                def emit_Supd(j):
                    a, b = 2 * j, 2 * j + 1
                    for ci in range(CT):
                        sd = PSG.tile([128, C], f32, name="big")
                        mm(sd[:], lhsT=Kn[a][:, ts(ci, 128)], rhs=Wt[a][:],
                           start=True, stop=False)
                        mm(sd[:], lhsT=Kn[b][:, ts(ci, 128)], rhs=Wt[b][:],
                           start=False, stop=True)
                        nc.vector.tensor_tensor(
                            S_f32[ci][:], sd[:], S_f32[ci][:], OP.add
                        )
                        nc.scalar.activation(S_sb[ci][:], S_f32[ci][:], AF.Identity)

                def do_pair(j, fill=None):
                    a, b = 2 * j, 2 * j + 1
                    vps_a = emit_V(a)
                    pps_a = emit_P_S(a, close=True) if j else None
                    vps_b = emit_V(b)
                    nBl = emit_gram(j + 1) if j < NP - 1 else None
                    if fill:
                        fill(0)
                    rhs_a = emit_combine(a, vps_a, pps_a)
                    emit_W(a, rhs_a)
                    pps_b = emit_P_S(b, close=False) if j else None
                    if j < NP - 1:
                        emit_constr(2 * j + 2, nBl[0])
                    if fill:
                        fill(1)
                    # cross term: P_b += gx^T W_a (closes / forms P_b group)
                    if pps_b is None:
                        pps_b = PSG.tile([128, C], f32, name="big")
                        mm(pps_b[:], lhsT=GX[j][:], rhs=Wt[a][:],
                           start=True, stop=True)
                    else:
                        mm(pps_b[:], lhsT=GX[j][:], rhs=Wt[a][:],
                           start=False, stop=True)
                    rhs_b = emit_combine(b, vps_b, pps_b)
                    emit_W(b, rhs_b)
                    if j < NP - 1:
                        emit_constr(2 * j + 3, nBl[1])
                    if fill:
                        fill(2)
                    if j < NP - 1:
                        emit_Supd(j)

                with tc.tile_pool(name="psC2", bufs=2, space="PSUM") as PSC2, \
                     tc.tile_pool(name="psC1", bufs=1, space="PSUM") as PSC1, \
                     tc.tile_pool(name="psCT", bufs=1, space="PSUM") as PSCT:

                    def emit_gram(j):
                        # pair grams: GA = [G_aa | gx], GB = G_bb; a=2j
                        a, b = 2 * j, 2 * j + 1
                        ga = PSC2.tile([128, 384], f32, name="c2")
                        for ci in range(CT):
                            mm(ga[:, 0:256], lhsT=KTr[ci][:, ts(a, 128)],
                               rhs=KTr[ci][:, ds(a * 128, 256)],
                               start=(ci == 0), stop=(ci == 3))
                        gb = PSC1.tile([128, 128], f32, name="c1")
                        for ci in range(CT):
                            mm(gb[:], lhsT=KTr[ci][:, ts(b, 128)],
                               rhs=KTr[ci][:, ts(b, 128)],
                               start=(ci == 0), stop=(ci == 3))
                        nc.scalar.activation(GX[j][:], ga[:, 128:256], AF.Identity)
                        # B = -tril(G,-1) * rn2 (rows): one fused DVE op each
                        Bl_a = SBC.tile([128, 128], bf16, name="Bla")
                        nc.vector.scalar_tensor_tensor(
                            Bl_a[:], ga[:, 0:128], rn2[:, a:a + 1], maskLn[:],
                            OP.mult, OP.mult,
                        )
                        Bl_b = SBC.tile([128, 128], bf16, name="Blb")
                        nc.vector.scalar_tensor_tensor(
                            Bl_b[:], gb[:], rn2[:, b:b + 1], maskLn[:],
                            OP.mult, OP.mult,
                        )
                        return Bl_a, Bl_b

                    def emit_constr(i, Bl):
                        # Mu[i] = (F1 + F2 + F1@F2)^T, F1 = B+B2+B3,
                        # F2 = B4+B8 (exact Neumann through degree 11).
                        # T1 = [Bu | B2u | B2l] lets [B3u | B4u] come from a
                        # single free-256 matmul with stationary B2l.
                        tps = PSCT.tile([128, 128], bf16, name="c1t")
                        nc.tensor.transpose(tps[:], Bl[:], ident_bf[:])
                        T1 = SBC.tile([128, 384], bf16, name="T1")
                        nc.scalar.activation(T1[:, 0:128], tps[:], AF.Identity)
                        Bu = T1[:, 0:128]

                        p1 = PSC2.tile([128, 384], f32, name="c2")
                        mm(p1[:, 0:128], lhsT=Bl[:], rhs=Bu, start=True, stop=True)
                        mm(p1[:, 128:256], lhsT=Bu, rhs=Bl[:], start=True, stop=True)
                        nc.scalar.activation(T1[:, 128:384], p1[:, 0:256], AF.Identity)
                        B2u, B2l = T1[:, 128:256], T1[:, 256:384]

                        pX = PSC2.tile([128, 384], f32, name="c2")
                        mm(pX[:, 0:256], lhsT=B2l, rhs=T1[:, 0:256],
                           start=True, stop=True)  # [B3u | B4u]
                        mm(pX[:, 256:384], lhsT=B2u, rhs=B2l, start=True, stop=True)
                        X = SBC.tile([128, 384], bf16, name="X")
                        nc.scalar.activation(X[:], pX[:], AF.Identity)
                        B3u, B4u, B4l = X[:, 0:128], X[:, 128:256], X[:, 256:384]

                        p3 = PSC2.tile([128, 384], f32, name="c2")
                        mm(p3[:, 0:128], lhsT=B4l, rhs=B4u, start=True, stop=True)
                        mm(p3[:, 128:256], lhsT=B4u, rhs=B4l, start=True, stop=True)
                        B8 = SBC.tile([128, 256], bf16, name="B8")
                        nc.scalar.activation(B8[:], p3[:, 0:256], AF.Identity)

                        # F1u = Bu + B2u + B3u; mp = (F1 F2)^T = F2u @ F1u;
                        # Mu = mp + F1u + F2u (adds done on DVE, not PE)
                        bb2 = SBC.tile([128, 128], bf16, name="bb2")
                        nc.vector.tensor_tensor(bb2[:], Bu, B2u, OP.add)
                        F1u = SBC.tile([128, 128], bf16, name="F1u")
                        nc.vector.tensor_tensor(F1u[:], B3u, bb2[:], OP.add)

                        F2 = SBC.tile([128, 256], bf16, name="F2")
                        nc.vector.tensor_tensor(F2[:], B8[:], X[:, 128:384], OP.add)
                        F2u, F2l = F2[:, 0:128], F2[:, 128:256]
                        fsum = SBC.tile([128, 128], bf16, name="fsum")
                        nc.vector.tensor_tensor(fsum[:], F1u[:], F2u, OP.add)

                        mp = PSC1.tile([128, 128], f32, name="c1")
                        mm(mp[:], lhsT=F2l, rhs=F1u[:], start=True, stop=True)
                        nc.vector.tensor_tensor(Mu[i][:], mp[:], fsum[:], OP.add)

                    # prologue: pair-0 construction
                    Bl_a, Bl_b = emit_gram(0)
                    emit_constr(0, Bl_a)
                    emit_constr(1, Bl_b)
                    for j in range(NP - 1):
                        do_pair(j)

                if kcut == "C":
                    do_pair(NP - 1)
                    with tc.tile_pool(name="sbX", bufs=2) as SBX:
                        for tj in range(NCH):
                            zt = SBX.tile([128, C], f32, name="zt")
                            nc.vector.tensor_copy(zt[:], Wt[tj][:])
                            nc.sync.dma_start(out=out[ts(tj, 128), :], in_=zt[:])
                else:
                    # ============== phase D: outputs =========================
                    with tc.tile_pool(name="psKV", bufs=1, space="PSUM") as PSKV:
                        kvt_ps = [PSKV.tile([128, C], f32, name=f"kvt{vi}")
                                  for vi in range(CT)]

                        def emit_kv(i0, i1):
                            for i in range(i0, i1):
                                for vi in range(CT):
                                    mm(kvt_ps[vi][:],
                                       lhsT=lru[i][:, ts(vi, 128)],
                                       rhs=Kn[i][:],
                                       start=(i == 0), stop=(i == NCH - 1))

                        emit_kv(0, 11)
                        do_pair(NP - 1, fill=lambda s: emit_kv(*[(11, 12), (12, 13),
                                                                 (13, 14)][s]))
                        emit_kv(14, NCH)
                        for vi in range(CT):
                            nc.any.tensor_copy(KVT[vi][:], kvt_ps[vi][:])

                    with tc.tile_pool(name="psD", bufs=2, space="PSUM") as PSD, \
                         tc.tile_pool(name="psDq", bufs=1, space="PSUM") as PSDQ:
                        for ki in range(CT):
                            kwp = PSD.tile([128, C], f32, name="dps")
                            for vi in range(CT):
                                mm(kwp[:], lhsT=KVT[vi][:, ts(ki, 128)],
                                   rhs=WoT[vi][:],
                                   start=(vi == 0), stop=(vi == 3))
                            nc.any.tensor_copy(KVW[ki][:], kwp[:])

                        # fold Q into the output: R = Wgq^T @ KVW,
                        # brow = bgq @ KVW + bo, out = x @ R + brow.
                        bqp = PSDQ.tile([1, C], f32, name="bqp")
                        for ri in range(CT):
                            mm(bqp[:], lhsT=bgq_sb[:, ri:ri + 1], rhs=KVW[ri][:],
                               start=(ri == 0), stop=(ri == 3))
                        brow = SBD.tile([1, C], bf16, name="brow")
                        nc.vector.tensor_tensor(brow[:], bqp[:], bo_sb[:], OP.add)
                        for ci in range(CT):
                            rps = PSD.tile([128, C], f32, name="dps")
                            for ri in range(CT):
                                mm(rps[:], lhsT=Wq[ri][:, ts(ci, 128)],
                                   rhs=KVW[ri][:],
                                   start=(ri == 0), stop=(ri == 3))
                            nc.any.tensor_copy(Rt[ci][:], rps[:])
                        b2p = PSD.tile([128, C], f32, name="dps")
                        mm(b2p[:], lhsT=ones_bf[:], rhs=brow[:], start=True, stop=True)
                        nc.any.tensor_copy(bo2_b[:], b2p[:])

                        for tj in range(NCH):
                            zps = PSD.tile([128, C], f32, name="dps")
                            for ci in range(CT):
                                mm(zps[:], lhsT=xT[ci][:, ts(tj, 128)],
                                   rhs=Rt[ci][:],
                                   start=(ci == 0), stop=(ci == 3))
                            zt = SBD.tile([128, C], f32, name="zt")
                            nc.vector.tensor_tensor(zt[:], zps[:], bo2_b[:], OP.add)
                            nc.sync.dma_start(out=out[ts(tj, 128), :], in_=zt[:])

                    if dbg:
                        for i in range(NCH):
                            dw = SBD.tile([128, C], f32, name="du")
                            nc.vector.tensor_copy(dw[:], Wt[i][:])
                            nc.sync.dma_start(out=dbg_W[ts(i, 128), :], in_=dw[:])
                            dk = SBD.tile([128, C], f32, name="du")
                            nc.vector.tensor_copy(dk[:], Kn[i][:])
                            nc.sync.dma_start(out=dbg_K[ts(i, 128), :], in_=dk[:])
                        for ci in range(CT):
                            dsb = SBD.tile([128, C], f32, name="du")
                            nc.vector.tensor_copy(dsb[:], S_f32[ci][:])
                            nc.sync.dma_start(out=dbg_S[ts(ci, 128), :], in_=dsb[:])
                            dkv = SBD.tile([128, C], f32, name="du")
                            nc.vector.tensor_copy(dkv[:], KVT[ci][:])
                            nc.sync.dma_start(out=dbg_KVT[ts(ci, 128), :], in_=dkv[:])

    nc.finalize()
    return nc


def _get_nc():
    if "nc" not in _CACHE:
        _CACHE["nc"] = _build()
    return _CACHE["nc"]


def _in_maps(inputs):
    def f(a):
        return np.ascontiguousarray(np.asarray(a, dtype=np.float32))

    x = f(inputs["x"])
    shared = {k: f(inputs[k]) for k in ("Wg", "bg", "Wlr", "blr", "Wo", "bo")}
    return [{"x": x[i], **shared} for i in range(N)]


def _run(in_maps, **kw):
    from concourse.bass_utils import run_bass_kernel_spmd

    nc = _get_nc()
    return run_bass_kernel_spmd(nc, in_maps, list(range(N)), **kw)


def kernel(**inputs) -> np.ndarray:
    res = _run(_in_maps(inputs))
    return np.stack([res.results[i]["out"] for i in range(N)]).astype(np.float32)


# revision 43
# speedup vs baseline: 1.0498x; 1.0090x over previous
# Bass/Trainium2 kernel for nn_Delta (DeltaNet-style recurrence).
#
# Problem (hardcoded): N=8, T=2048, C=512, fp32 I/O.
#   g = x @ Wg.T + bg ; q,k,v = split(g) ; lr = x @ Wlr.T + blr
#   khat = k / ||k||
#   delta-rule scan:  u_t = v_t - khat_t @ S ; S += outer(khat_t, u_t)
#   kv = sum_t khat_t (x) (lr_t * u_t) ; y = q @ kv ; out = y @ Wo.T + bo
#
# Sharding: data-parallel over N across the 8 cores (sample i -> core i),
# weights replicated. No collectives.
#
# Per-core algorithm: chunked parallel delta rule, chunk L=128, in
# "W-space": with D = diag(1/||k_raw||) per chunk and W = D^-1-free
# substitution W = D u, the recurrence becomes
#   (I + D^2 G) W = D V - D^2 (Kraw S + cross),   G = tril(Kraw Kraw^T, -1)
#   S += Kraw^T W,   kv = Kraw^T (lr . W)
# so only RAW projections appear in matmuls; the 1/||k|| and 1/||k||^2
# row scalings ride on PSUM evacuations (per-partition scale APs).
#
# (I + B)^-1 with B = -D^2 G is truncated exactly through degree 11 via
# the two-factor form (I+F1)(I+F2), F1 = B+B^2+B^3, F2 = B^4+B^8,
# applied merged: W = rhs + M' rhs with M' = F1+F2+F1F2  (measured
# truncation error 2.3e-3 in f64 on this data, far below the bf16 noise).
#
# Q is never materialized: y = q @ kv folds into out = x @ R + brow with
# R = Wgq^T (kv Wo^T) and brow = bgq (kv Wo^T) + bo, reusing resident xT.
#
# No DMA transposes: x, Wg, Wo are cast-DMA'd naturally and transposed
# on the tensor engine (transpose mode), as is Bl -> Bu.

import os

import numpy as np

N, T, C = 8, 2048, 512
L = 128
NCH = T // L  # 16 chunks
NP = NCH // 2  # 8 chunk pairs
CT = C // 128  # 4 c-tiles

_CACHE = {}


def _build():
    import concourse.bacc as bacc
    import concourse.mybir as mybir
    import concourse.tile as tile
    from concourse.bass import ts, ds
    from concourse.masks import make_identity, make_lower_triangular

    f32 = mybir.dt.float32
    bf16 = mybir.dt.bfloat16
    AF = mybir.ActivationFunctionType
    OP = mybir.AluOpType

    nc = bacc.Bacc("TRN2")
    x = nc.declare_dram_parameter("x", [T, C], f32, isOutput=False)
    Wg = nc.declare_dram_parameter("Wg", [3 * C, C], f32, isOutput=False)
    bg = nc.declare_dram_parameter("bg", [3 * C], f32, isOutput=False)
    Wlr = nc.declare_dram_parameter("Wlr", [1, C], f32, isOutput=False)
    blr = nc.declare_dram_parameter("blr", [1], f32, isOutput=False)
    Wo = nc.declare_dram_parameter("Wo", [C, C], f32, isOutput=False)
    bo = nc.declare_dram_parameter("bo", [C], f32, isOutput=False)
    out = nc.declare_dram_parameter("out", [T, C], f32, isOutput=True)
    dbg = os.environ.get("KDBG") == "1"
    if dbg:
        dbg_W = nc.declare_dram_parameter("dbg_W", [T, C], f32, isOutput=True)
        dbg_K = nc.declare_dram_parameter("dbg_K", [T, C], f32, isOutput=True)
        dbg_S = nc.declare_dram_parameter("dbg_S", [C, C], f32, isOutput=True)
        dbg_KVT = nc.declare_dram_parameter("dbg_KVT", [C, C], f32, isOutput=True)

    mm = nc.tensor.matmul

    with tile.TileContext(nc) as tc:
        with tc.tile_pool(name="persist", bufs=1) as P:
            # ---- constants / small tensors ----
            maskLn = P.tile([128, 128], f32, name="maskLn")
            make_lower_triangular(nc, maskLn[:], val=-1.0, diag=False)
            ones_bf = P.tile([1, 128], bf16, name="ones_bf")
            nc.vector.memset(ones_bf[:], 1.0)
            ident_bf = P.tile([128, 128], bf16, name="ident_bf")
            make_identity(nc, ident_bf[:])

            bgk_sb = P.tile([1, C], bf16, name="bgk_sb")
            bgv_sb = P.tile([1, C], bf16, name="bgv_sb")
            bo_sb = P.tile([1, C], bf16, name="bo_sb")
            bgq_sb = P.tile([128, CT], bf16, name="bgq_sb")
            WlrT_sb = P.tile([128, CT], bf16, name="WlrT_sb")
            blr_sb = P.tile([1, 1], f32, name="blr_sb")

            def load_small():
                nc.gpsimd.dma_start(out=bgk_sb[:], in_=bg[C:2 * C])
                nc.gpsimd.dma_start(out=bgv_sb[:], in_=bg[2 * C:3 * C])
                nc.gpsimd.dma_start(out=bo_sb[:], in_=bo[:])
                nc.gpsimd.dma_start(
                    out=bgq_sb[:], in_=bg[0:C].rearrange("(i p) -> p i", p=128)
                )
                nc.gpsimd.dma_start(
                    out=WlrT_sb[:], in_=Wlr[0, :].rearrange("(i p) -> p i", p=128)
                )
                nc.gpsimd.dma_start(out=blr_sb[:], in_=blr[:])

            # ---- persistent tensors ----
            xT = [P.tile([128, T], bf16, name=f"xT{i}") for i in range(CT)]
            WgT = [P.tile([128, 3 * C], bf16, name=f"WgT{i}") for i in range(CT)]
            WoT = [P.tile([128, C], bf16, name=f"WoT{i}") for i in range(CT)]
            KTr = [P.tile([128, T], bf16, name=f"KTr{i}") for i in range(CT)]
            Kn = [P.tile([128, C], bf16, name=f"Kn{i}") for i in range(NCH)]
            Wt = [P.tile([128, C], bf16, name=f"Wt{i}") for i in range(NCH)]
            lru = [P.tile([128, C], bf16, name=f"lru{i}") for i in range(NCH)]
            Wq = [P.tile([128, C], bf16, name=f"Wq{i}") for i in range(CT)]
            Rt = [P.tile([128, C], bf16, name=f"Rt{i}") for i in range(CT)]
            bo2_b = P.tile([128, C], f32, name="bo2_b")
            S_f32 = [P.tile([128, C], f32, name=f"Sf{i}") for i in range(CT)]
            S_sb = [P.tile([128, C], bf16, name=f"S{i}") for i in range(CT)]
            KVT = [P.tile([128, C], bf16, name=f"KVT{i}") for i in range(CT)]
            KVW = [P.tile([128, C], bf16, name=f"KVW{i}") for i in range(CT)]
            bgk_b = P.tile([128, C], bf16, name="bgk_b")
            bo_b = P.tile([128, C], f32, name="bo_b")
            lrT = P.tile([1, T], f32, name="lrT")
            lrn = P.tile([128, NCH], f32, name="lrn")
            n2_all = P.tile([128, NCH], f32, name="n2_all")
            rn2 = P.tile([128, NCH], f32, name="rn2")  # 1/||k||^2
            rn2n = P.tile([128, NCH], f32, name="rn2n")  # -1/||k||^2
            rn_all = P.tile([128, NCH], f32, name="rn_all")  # 1/||k||
            # per-chunk construction outputs (consumed next pair at latest)
            Mu = [P.tile([128, 128], bf16, name=f"Mu{i}") for i in range(NCH)]
            GX = [P.tile([128, 128], bf16, name=f"GX{i}") for i in range(NP)]

            # ============ phase A+B: loads, PE transposes, projections =======
            # Load order: x, Wg-k block, then the kps/KT chain runs on PE
            # while Wg-v, Wg-q, Wo stream in behind it.  Wg-q row tiles are
            # also kept in natural layout (Wq) for the output-side fold
            # y = x @ (Wgq^T kv Wo^T): Q is never materialized.
            with tc.tile_pool(name="stg", bufs=2) as STG, \
                 tc.tile_pool(name="sbB", bufs=4) as SBB, \
                 tc.tile_pool(name="psT", bufs=2, space="PSUM") as PST, \
                 tc.tile_pool(name="psB", bufs=2, space="PSUM") as PSB, \
                 tc.tile_pool(name="psKT", bufs=2, space="PSUM") as PSKT, \
                 tc.tile_pool(name="psL", bufs=1, space="PSUM") as PSL:

                def load_dma(src, row0):
                    grp = []
                    for jj in range(4):
                        t = STG.tile([128, C], bf16, name=f"stg{jj}")
                        nc.gpsimd.dma_start(
                            out=t[:],
                            in_=src[row0 + jj * 128:row0 + (jj + 1) * 128, :],
                        )
                        grp.append(t)
                    return grp

                def load_tp(grp, dstT, col0):
                    for ci in range(CT):
                        ps = PST.tile([128, 512], bf16, name="pst")
                        for jj in range(4):
                            nc.tensor.transpose(
                                ps[:, ts(jj, 128)],
                                grp[jj][:, ts(ci, 128)],
                                ident_bf[:],
                            )
                        nc.any.tensor_copy(dstT[ci][:, ds(col0, 512)], ps[:])

                def load_group(src, row0, dstT, col0):
                    load_tp(load_dma(src, row0), dstT, col0)

                load_group(x, 0, xT, 0)
                load_group(Wg, C, WgT, C)  # k rows
                load_small()
                load_group(x, 512, xT, 512)
                # bias broadcast rows -> [128, C] tiles (one matmul each)
                bps = PSL.tile([128, C], f32, name="bps")
                mm(bps[:], lhsT=ones_bf[:], rhs=bgk_sb[:], start=True, stop=True)
                nc.any.tensor_copy(bgk_b[:], bps[:])
                bps2 = PSL.tile([128, C], f32, name="bps")
                mm(bps2[:], lhsT=ones_bf[:], rhs=bo_sb[:], start=True, stop=True)
                nc.any.tensor_copy(bo_b[:], bps2[:])

                for tj in range(NCH):
                    if tj == 4:
                        load_group(x, 2 * 512, xT, 2 * 512)
                    elif tj == 6:
                        load_group(x, 3 * 512, xT, 3 * 512)
                    kps = PSB.tile([128, C], f32, name="kps")
                    for ci in range(CT):
                        mm(kps[:], lhsT=xT[ci][:, ts(tj, 128)],
                           rhs=WgT[ci][:, ds(C, C)],
                           start=(ci == 0), stop=(ci == 3))
                    # Kn = kps + bgk (broadcast tile); n2 = sum Kn^2
                    nc.vector.tensor_tensor(
                        Kn[tj][:], kps[:], bgk_b[:], OP.add
                    )
                    junk = SBB.tile([128, C], f32, name="junk")
                    nc.vector.scalar_tensor_tensor(
                        junk[:], Kn[tj][:], 1.0, Kn[tj][:], OP.mult, OP.mult,
                        accum_out=n2_all[:, tj:tj + 1],
                    )
                    if tj % 4 == 3:
                        for ci in range(CT):
                            ps = PSKT.tile([128, 512], bf16, name="pskt")
                            for jj in range(4):
                                nc.tensor.transpose(
                                    ps[:, ts(jj, 128)],
                                    Kn[tj - 3 + jj][:, ts(ci, 128)],
                                    ident_bf[:],
                                )
                            nc.any.tensor_copy(
                                KTr[ci][:, ds((tj - 3) * 128, 512)], ps[:]
                            )
                    if tj == 3:
                        load_group(Wg, 2 * C, WgT, 2 * C)  # v rows
                    elif tj == 7:
                        for jj in range(4):  # q rows, natural layout only
                            nc.gpsimd.dma_start(
                                out=Wq[jj][:], in_=Wg[jj * 128:(jj + 1) * 128, :]
                            )
                    elif tj == 11:
                        load_group(Wo, 0, WoT, 0)

                # row scalings
                nc.vector.reciprocal(rn2[:], n2_all[:])
                nc.vector.tensor_scalar_mul(rn2n[:], rn2[:], -1.0)
                nc.scalar.activation(rn_all[:], rn2[:], AF.Sqrt)

                # lr row: lrT[1, T] then scatter to lrn [128, NCH]
                for tg in range(4):
                    lps = PSL.tile([1, 512], f32, name="lps")
                    for ci in range(CT):
                        mm(lps[:], lhsT=WlrT_sb[:, ci:ci + 1],
                           rhs=xT[ci][:, ds(tg * 512, 512)],
                           start=(ci == 0), stop=(ci == 3))
                    nc.scalar.activation(
                        lrT[:, ds(tg * 512, 512)], lps[:], AF.Identity,
                        bias=blr_sb[:, 0:1], scale=1.0,
                    )
                for i in range(NCH):
                    nc.gpsimd.dma_start(
                        out=lrn[:, i:i + 1], in_=lrT[0:1, ts(i, 128)]
                    )

            kcut = os.environ.get("KCUT", "")
            if kcut == "B":
                with tc.tile_pool(name="sbX", bufs=2) as SBX:
                    for tj in range(NCH):
                        zt = SBX.tile([128, C], f32, name="zt")
                        nc.vector.tensor_copy(zt[:], Kn[tj][:])
                        nc.sync.dma_start(out=out[ts(tj, 128), :], in_=zt[:])

            # ================= phase C: delta-rule recurrence ================
            # Pool scoping: construction pools (c2/c1/c1t) close after pair
            # NP-2 (all Mu/GX are built one pair ahead), freeing their PSUM
            # banks for the kv accumulators, which run during pair NP-1's
            # stalls.  PSG then hands its banks to psD for the output chain.
            if kcut in ("B",):
                pass
            elif True:
              with tc.tile_pool(name="sbC", bufs=4) as SBC, \
                 tc.tile_pool(name="sbR", bufs=4) as SBR, \
                 tc.tile_pool(name="sbD", bufs=4) as SBD:
                import contextlib
                _psg = contextlib.ExitStack()
                PSG = _psg.enter_context(
                    tc.tile_pool(name="psBIG", bufs=4, space="PSUM")
                )
                for ci in range(CT):
                    nc.gpsimd.memset(S_f32[ci][:], 0.0)

                def emit_V(i):
                    vps = PSG.tile([128, C], f32, name="big")
                    for ci in range(CT):
                        mm(vps[:], lhsT=xT[ci][:, ts(i, 128)],
                           rhs=WgT[ci][:, ds(2 * C, C)],
                           start=(ci == 0), stop=False)
                    mm(vps[:], lhsT=ones_bf[:], rhs=bgv_sb[:],
                       start=False, stop=True)
                    return vps

                def emit_P_S(i, close):
                    # P = Kraw_i S0 (+ cross term appended later for odd i)
                    pps = PSG.tile([128, C], f32, name="big")
                    for ci in range(CT):
                        mm(pps[:], lhsT=KTr[ci][:, ts(i, 128)], rhs=S_sb[ci][:],
                           start=(ci == 0), stop=(close and ci == 3))
                    return pps

                def emit_combine(i, vps, pps):
                    # rhs = rn * V  +  (-rn2) * P  (both legs on DVE so the
                    # chain has no cross-engine hop)
                    e1 = SBR.tile([128, C], bf16, name="e1")
                    nc.scalar.activation(
                        e1[:], vps[:], AF.Identity, scale=rn_all[:, i:i + 1]
                    )
                    if pps is None:
                        return e1
                    rhs = SBR.tile([128, C], bf16, name="rhs")
                    nc.vector.scalar_tensor_tensor(
                        rhs[:], pps[:], rn2n[:, i:i + 1], e1[:],
                        OP.mult, OP.add,
                    )
                    return rhs

                def emit_W(i, rhs):
                    wps = PSG.tile([128, C], f32, name="big")
                    mm(wps[:], lhsT=Mu[i][:], rhs=rhs[:], start=True, stop=True)
                    nc.vector.tensor_tensor(Wt[i][:], wps[:], rhs[:], OP.add)
                    nc.vector.tensor_scalar_mul(lru[i][:], Wt[i][:], lrn[:, i:i + 1])

                def emit_Supd(j):
                    a, b = 2 * j, 2 * j + 1
                    for ci in range(CT):
                        sd = PSG.tile([128, C], f32, name="big")
                        mm(sd[:], lhsT=Kn[a][:, ts(ci, 128)], rhs=Wt[a][:],
                           start=True, stop=False)
                        mm(sd[:], lhsT=Kn[b][:, ts(ci, 128)], rhs=Wt[b][:],
                           start=False, stop=True)
                        nc.vector.tensor_tensor(
                            S_f32[ci][:], sd[:], S_f32[ci][:], OP.add
                        )
                        nc.scalar.activation(S_sb[ci][:], S_f32[ci][:], AF.Identity)

                def do_pair(j, fill=None):
                    a, b = 2 * j, 2 * j + 1
                    vps_a = emit_V(a)
                    pps_a = emit_P_S(a, close=True) if j else None
                    vps_b = emit_V(b)
                    nBl = emit_gram(j + 1) if j < NP - 1 else None
                    if fill:
                        fill(0)
                    rhs_a = emit_combine(a, vps_a, pps_a)
                    emit_W(a, rhs_a)
                    pps_b = emit_P_S(b, close=False) if j else None
                    if j < NP - 1:
                        emit_constr(2 * j + 2, nBl[0])
                    if fill:
                        fill(1)
                    # cross term: P_b += gx^T W_a (closes / forms P_b group)
                    if pps_b is None:
                        pps_b = PSG.tile([128, C], f32, name="big")
                        mm(pps_b[:], lhsT=GX[j][:], rhs=Wt[a][:],
                           start=True, stop=True)
                    else:
                        mm(pps_b[:], lhsT=GX[j][:], rhs=Wt[a][:],
                           start=False, stop=True)
                    rhs_b = emit_combine(b, vps_b, pps_b)
                    emit_W(b, rhs_b)
                    if j < NP - 1:
                        emit_constr(2 * j + 3, nBl[1])
                    if fill:
                        fill(2)
                    if j < NP - 1:
                        emit_Supd(j)

                with tc.tile_pool(name="psC2", bufs=2, space="PSUM") as PSC2, \
                     tc.tile_pool(name="psC1", bufs=1, space="PSUM") as PSC1, \
                     tc.tile_pool(name="psCT", bufs=1, space="PSUM") as PSCT:

                    def emit_gram(j):
                        # pair grams: GA = [G_aa | gx], GB = G_bb; a=2j
                        a, b = 2 * j, 2 * j + 1
                        ga = PSC2.tile([128, 384], f32, name="c2")
                        for ci in range(CT):
                            mm(ga[:, 0:256], lhsT=KTr[ci][:, ts(a, 128)],
                               rhs=KTr[ci][:, ds(a * 128, 256)],
                               start=(ci == 0), stop=(ci == 3))
                        gb = PSC1.tile([128, 128], f32, name="c1")
                        for ci in range(CT):
                            mm(gb[:], lhsT=KTr[ci][:, ts(b, 128)],
                               rhs=KTr[ci][:, ts(b, 128)],
                               start=(ci == 0), stop=(ci == 3))
                        nc.scalar.activation(GX[j][:], ga[:, 128:256], AF.Identity)
                        # B = -tril(G,-1) * rn2 (rows): one fused DVE op each
                        Bl_a = SBC.tile([128, 128], bf16, name="Bla")
                        nc.vector.scalar_tensor_tensor(
                            Bl_a[:], ga[:, 0:128], rn2[:, a:a + 1], maskLn[:],
                            OP.mult, OP.mult,
                        )
                        Bl_b = SBC.tile([128, 128], bf16, name="Blb")
                        nc.vector.scalar_tensor_tensor(
                            Bl_b[:], gb[:], rn2[:, b:b + 1], maskLn[:],
                            OP.mult, OP.mult,
                        )
                        return Bl_a, Bl_b

                    def emit_constr(i, Bl):
                        # Mu[i] = (F1 + F2 + F1@F2)^T, F1 = B+B2+B3,
                        # F2 = B4+B8 (exact Neumann through degree 11).
                        # T1 = [Bu | B2u | B2l] lets [B3u | B4u] come from a
                        # single free-256 matmul with stationary B2l.
                        tps = PSCT.tile([128, 128], bf16, name="c1t")
                        nc.tensor.transpose(tps[:], Bl[:], ident_bf[:])
                        T1 = SBC.tile([128, 384], bf16, name="T1")
                        nc.scalar.activation(T1[:, 0:128], tps[:], AF.Identity)
                        Bu = T1[:, 0:128]

                        p1 = PSC2.tile([128, 384], f32, name="c2")
                        mm(p1[:, 0:128], lhsT=Bl[:], rhs=Bu, start=True, stop=True)
                        mm(p1[:, 128:256], lhsT=Bu, rhs=Bl[:], start=True, stop=True)
                        nc.scalar.activation(T1[:, 128:384], p1[:, 0:256], AF.Identity)
                        B2u, B2l = T1[:, 128:256], T1[:, 256:384]

                        pX = PSC2.tile([128, 384], f32, name="c2")
                        mm(pX[:, 0:256], lhsT=B2l, rhs=T1[:, 0:256],
                           start=True, stop=True)  # [B3u | B4u]
                        mm(pX[:, 256:384], lhsT=B2u, rhs=B2l, start=True, stop=True)
                        X = SBC.tile([128, 384], bf16, name="X")
                        nc.scalar.activation(X[:], pX[:], AF.Identity)
                        B3u, B4u, B4l = X[:, 0:128], X[:, 128:256], X[:, 256:384]

                        p3 = PSC2.tile([128, 384], f32, name="c2")
                        mm(p3[:, 0:128], lhsT=B4l, rhs=B4u, start=True, stop=True)
                        mm(p3[:, 128:256], lhsT=B4u, rhs=B4l, start=True, stop=True)
                        B8 = SBC.tile([128, 256], bf16, name="B8")
                        nc.scalar.activation(B8[:], p3[:, 0:256], AF.Identity)

                        # F1u = Bu + B2u + B3u; mp = (F1 F2)^T = F2u @ F1u;
                        # Mu = mp + F1u + F2u (adds done on DVE, not PE)
                        bb2 = SBC.tile([128, 128], bf16, name="bb2")
                        nc.vector.tensor_tensor(bb2[:], Bu, B2u, OP.add)
                        F1u = SBC.tile([128, 128], bf16, name="F1u")
                        nc.vector.tensor_tensor(F1u[:], B3u, bb2[:], OP.add)

                        F2 = SBC.tile([128, 256], bf16, name="F2")
                        nc.vector.tensor_tensor(F2[:], B8[:], X[:, 128:384], OP.add)
                        F2u, F2l = F2[:, 0:128], F2[:, 128:256]
                        fsum = SBC.tile([128, 128], bf16, name="fsum")
                        nc.vector.tensor_tensor(fsum[:], F1u[:], F2u, OP.add)

                        mp = PSC1.tile([128, 128], f32, name="c1")
                        mm(mp[:], lhsT=F2l, rhs=F1u[:], start=True, stop=True)
                        nc.vector.tensor_tensor(Mu[i][:], mp[:], fsum[:], OP.add)

                    # prologue: pair-0 construction
                    Bl_a, Bl_b = emit_gram(0)
                    emit_constr(0, Bl_a)
                    emit_constr(1, Bl_b)
                    for j in range(NP - 1):
                        do_pair(j)

                if kcut == "C":
                    do_pair(NP - 1)
                    _psg.close()
                    with tc.tile_pool(name="sbX", bufs=2) as SBX:
                        for tj in range(NCH):
                            zt = SBX.tile([128, C], f32, name="zt")
                            nc.vector.tensor_copy(zt[:], Wt[tj][:])
                            nc.sync.dma_start(out=out[ts(tj, 128), :], in_=zt[:])
                else:
                    # ============== phase D: outputs =========================
                    with tc.tile_pool(name="psKV", bufs=1, space="PSUM") as PSKV:
                        kvt_ps = [PSKV.tile([128, C], f32, name=f"kvt{vi}")
                                  for vi in range(CT)]

                        def emit_kv(i0, i1):
                            for i in range(i0, i1):
                                for vi in range(CT):
                                    mm(kvt_ps[vi][:],
                                       lhsT=lru[i][:, ts(vi, 128)],
                                       rhs=Kn[i][:],
                                       start=(i == 0), stop=(i == NCH - 1))

                        emit_kv(0, 11)
                        do_pair(NP - 1, fill=lambda s: emit_kv(*[(11, 12), (12, 13),
                                                                 (13, 14)][s]))
                        emit_kv(14, NCH)
                        for vi in range(CT):
                            nc.any.tensor_copy(KVT[vi][:], kvt_ps[vi][:])

                    _psg.close()
                    with tc.tile_pool(name="psD", bufs=2, space="PSUM") as PSD, \
                         tc.tile_pool(name="psDq", bufs=1, space="PSUM") as PSDQ:
                        for ki in range(CT):
                            kwp = PSD.tile([128, C], f32, name="dps")
                            for vi in range(CT):
                                mm(kwp[:], lhsT=KVT[vi][:, ts(ki, 128)],
                                   rhs=WoT[vi][:],
                                   start=(vi == 0), stop=(vi == 3))
                            nc.any.tensor_copy(KVW[ki][:], kwp[:])

                        # fold Q into the output: R = Wgq^T @ KVW,
                        # brow = bgq @ KVW + bo, out = x @ R + brow.
                        bqp = PSDQ.tile([1, C], f32, name="bqp")
                        for ri in range(CT):
                            mm(bqp[:], lhsT=bgq_sb[:, ri:ri + 1], rhs=KVW[ri][:],
                               start=(ri == 0), stop=(ri == 3))
                        brow = SBD.tile([1, C], bf16, name="brow")
                        nc.vector.tensor_tensor(brow[:], bqp[:], bo_sb[:], OP.add)
                        for ci in range(CT):
                            rps = PSD.tile([128, C], f32, name="dps")
                            for ri in range(CT):
                                mm(rps[:], lhsT=Wq[ri][:, ts(ci, 128)],
                                   rhs=KVW[ri][:],
                                   start=(ri == 0), stop=(ri == 3))
                            nc.any.tensor_copy(Rt[ci][:], rps[:])
                        b2p = PSD.tile([128, C], f32, name="dps")
                        mm(b2p[:], lhsT=ones_bf[:], rhs=brow[:], start=True, stop=True)
                        nc.any.tensor_copy(bo2_b[:], b2p[:])

                        for tj in range(NCH):
                            zps = PSD.tile([128, C], f32, name="dps")
                            for ci in range(CT):
                                mm(zps[:], lhsT=xT[ci][:, ts(tj, 128)],
                                   rhs=Rt[ci][:],
                                   start=(ci == 0), stop=(ci == 3))
                            zt = SBD.tile([128, C], f32, name="zt")
                            nc.vector.tensor_tensor(zt[:], zps[:], bo2_b[:], OP.add)
                            nc.sync.dma_start(out=out[ts(tj, 128), :], in_=zt[:])

                    if dbg:
                        for i in range(NCH):
                            dw = SBD.tile([128, C], f32, name="du")
                            nc.vector.tensor_copy(dw[:], Wt[i][:])
                            nc.sync.dma_start(out=dbg_W[ts(i, 128), :], in_=dw[:])
                            dk = SBD.tile([128, C], f32, name="du")
                            nc.vector.tensor_copy(dk[:], Kn[i][:])
                            nc.sync.dma_start(out=dbg_K[ts(i, 128), :], in_=dk[:])
                        for ci in range(CT):
                            dsb = SBD.tile([128, C], f32, name="du")
                            nc.vector.tensor_copy(dsb[:], S_f32[ci][:])
                            nc.sync.dma_start(out=dbg_S[ts(ci, 128), :], in_=dsb[:])
                            dkv = SBD.tile([128, C], f32, name="du")
                            nc.vector.tensor_copy(dkv[:], KVT[ci][:])
                            nc.sync.dma_start(out=dbg_KVT[ts(ci, 128), :], in_=dkv[:])

    nc.finalize()
    return nc


def _get_nc():
    if "nc" not in _CACHE:
        _CACHE["nc"] = _build()
    return _CACHE["nc"]


def _in_maps(inputs):
    def f(a):
        return np.ascontiguousarray(np.asarray(a, dtype=np.float32))

    x = f(inputs["x"])
    shared = {k: f(inputs[k]) for k in ("Wg", "bg", "Wlr", "blr", "Wo", "bo")}
    return [{"x": x[i], **shared} for i in range(N)]


def _run(in_maps, **kw):
    from concourse.bass_utils import run_bass_kernel_spmd

    nc = _get_nc()
    return run_bass_kernel_spmd(nc, in_maps, list(range(N)), **kw)


def kernel(**inputs) -> np.ndarray:
    res = _run(_in_maps(inputs))
    return np.stack([res.results[i]["out"] for i in range(N)]).astype(np.float32)


# revision 44
# speedup vs baseline: 1.0523x; 1.0025x over previous
# Bass/Trainium2 kernel for nn_Delta (DeltaNet-style recurrence).
#
# Problem (hardcoded): N=8, T=2048, C=512, fp32 I/O.
#   g = x @ Wg.T + bg ; q,k,v = split(g) ; lr = x @ Wlr.T + blr
#   khat = k / ||k||
#   delta-rule scan:  u_t = v_t - khat_t @ S ; S += outer(khat_t, u_t)
#   kv = sum_t khat_t (x) (lr_t * u_t) ; y = q @ kv ; out = y @ Wo.T + bo
#
# Sharding: data-parallel over N across the 8 cores (sample i -> core i),
# weights replicated. No collectives.
#
# Per-core algorithm: chunked parallel delta rule, chunk L=128, in
# "W-space": with D = diag(1/||k_raw||) per chunk and W = D^-1-free
# substitution W = D u, the recurrence becomes
#   (I + D^2 G) W = D V - D^2 (Kraw S + cross),   G = tril(Kraw Kraw^T, -1)
#   S += Kraw^T W,   kv = Kraw^T (lr . W)
# so only RAW projections appear in matmuls; the 1/||k|| and 1/||k||^2
# row scalings ride on PSUM evacuations (per-partition scale APs).
#
# (I + B)^-1 with B = -D^2 G is truncated exactly through degree 11 via
# the two-factor form (I+F1)(I+F2), F1 = B+B^2+B^3, F2 = B^4+B^8,
# applied merged: W = rhs + M' rhs with M' = F1+F2+F1F2  (measured
# truncation error 2.3e-3 in f64 on this data, far below the bf16 noise).
#
# Q is never materialized: y = q @ kv folds into out = x @ R + brow with
# R = Wgq^T (kv Wo^T) and brow = bgq (kv Wo^T) + bo, reusing resident xT.
#
# No DMA transposes: x, Wg, Wo are cast-DMA'd naturally and transposed
# on the tensor engine (transpose mode), as is Bl -> Bu.

import os

import numpy as np

N, T, C = 8, 2048, 512
L = 128
NCH = T // L  # 16 chunks
NP = NCH // 2  # 8 chunk pairs
CT = C // 128  # 4 c-tiles

_CACHE = {}


def _build():
    import concourse.bacc as bacc
    import concourse.mybir as mybir
    import concourse.tile as tile
    from concourse.bass import ts, ds
    from concourse.masks import make_identity, make_lower_triangular

    f32 = mybir.dt.float32
    bf16 = mybir.dt.bfloat16
    AF = mybir.ActivationFunctionType
    OP = mybir.AluOpType

    nc = bacc.Bacc("TRN2")
    x = nc.declare_dram_parameter("x", [T, C], f32, isOutput=False)
    Wg = nc.declare_dram_parameter("Wg", [3 * C, C], f32, isOutput=False)
    bg = nc.declare_dram_parameter("bg", [3 * C], f32, isOutput=False)
    Wlr = nc.declare_dram_parameter("Wlr", [1, C], f32, isOutput=False)
    blr = nc.declare_dram_parameter("blr", [1], f32, isOutput=False)
    Wo = nc.declare_dram_parameter("Wo", [C, C], f32, isOutput=False)
    bo = nc.declare_dram_parameter("bo", [C], f32, isOutput=False)
    out = nc.declare_dram_parameter("out", [T, C], f32, isOutput=True)
    dbg = os.environ.get("KDBG") == "1"
    if dbg:
        dbg_W = nc.declare_dram_parameter("dbg_W", [T, C], f32, isOutput=True)
        dbg_K = nc.declare_dram_parameter("dbg_K", [T, C], f32, isOutput=True)
        dbg_S = nc.declare_dram_parameter("dbg_S", [C, C], f32, isOutput=True)
        dbg_KVT = nc.declare_dram_parameter("dbg_KVT", [C, C], f32, isOutput=True)

    mm = nc.tensor.matmul

    with tile.TileContext(nc) as tc:
        with tc.tile_pool(name="persist", bufs=1) as P:
            # ---- constants / small tensors ----
            maskLn = P.tile([128, 128], f32, name="maskLn")
            make_lower_triangular(nc, maskLn[:], val=-1.0, diag=False)
            ones_bf = P.tile([1, 128], bf16, name="ones_bf")
            nc.vector.memset(ones_bf[:], 1.0)
            ident_bf = P.tile([128, 128], bf16, name="ident_bf")
            make_identity(nc, ident_bf[:])

            bgk_sb = P.tile([1, C], bf16, name="bgk_sb")
            bgv_sb = P.tile([1, C], bf16, name="bgv_sb")
            bo_sb = P.tile([1, C], bf16, name="bo_sb")
            bgq_sb = P.tile([128, CT], bf16, name="bgq_sb")
            WlrT_sb = P.tile([128, CT], bf16, name="WlrT_sb")
            blr_sb = P.tile([1, 1], f32, name="blr_sb")

            def load_small():
                nc.gpsimd.dma_start(out=bgk_sb[:], in_=bg[C:2 * C])
                nc.gpsimd.dma_start(out=bgv_sb[:], in_=bg[2 * C:3 * C])
                nc.gpsimd.dma_start(out=bo_sb[:], in_=bo[:])
                nc.gpsimd.dma_start(
                    out=bgq_sb[:], in_=bg[0:C].rearrange("(i p) -> p i", p=128)
                )
                nc.gpsimd.dma_start(
                    out=WlrT_sb[:], in_=Wlr[0, :].rearrange("(i p) -> p i", p=128)
                )
                nc.gpsimd.dma_start(out=blr_sb[:], in_=blr[:])

            # ---- persistent tensors ----
            xT = [P.tile([128, T], bf16, name=f"xT{i}") for i in range(CT)]
            WgT = [P.tile([128, 3 * C], bf16, name=f"WgT{i}") for i in range(CT)]
            WoT = [P.tile([128, C], bf16, name=f"WoT{i}") for i in range(CT)]
            KTr = [P.tile([128, T], bf16, name=f"KTr{i}") for i in range(CT)]
            Kn = [P.tile([128, C], bf16, name=f"Kn{i}") for i in range(NCH)]
            Wt = [P.tile([128, C], bf16, name=f"Wt{i}") for i in range(NCH)]
            lru = [P.tile([128, C], bf16, name=f"lru{i}") for i in range(NCH)]
            Wq = [P.tile([128, C], bf16, name=f"Wq{i}") for i in range(CT)]
            Rt = [P.tile([128, C], bf16, name=f"Rt{i}") for i in range(CT)]
            bo2_b = P.tile([128, C], f32, name="bo2_b")
            S_f32 = [P.tile([128, C], f32, name=f"Sf{i}") for i in range(CT)]
            S_sb = [P.tile([128, C], bf16, name=f"S{i}") for i in range(CT)]
            KVT = [P.tile([128, C], bf16, name=f"KVT{i}") for i in range(CT)]
            KVW = [P.tile([128, C], bf16, name=f"KVW{i}") for i in range(CT)]
            bgk_b = P.tile([128, C], bf16, name="bgk_b")
            bo_b = P.tile([128, C], f32, name="bo_b")
            lrT = P.tile([1, T], f32, name="lrT")
            lrn = P.tile([128, NCH], f32, name="lrn")
            n2_all = P.tile([128, NCH], f32, name="n2_all")
            rn2 = P.tile([128, NCH], f32, name="rn2")  # 1/||k||^2
            rn2n = P.tile([128, NCH], f32, name="rn2n")  # -1/||k||^2
            rn_all = P.tile([128, NCH], f32, name="rn_all")  # 1/||k||
            # per-chunk construction outputs (consumed next pair at latest)
            Mu = [P.tile([128, 128], bf16, name=f"Mu{i}") for i in range(NCH)]
            GX = [P.tile([128, 128], bf16, name=f"GX{i}") for i in range(NP)]

            # ============ phase A+B: loads, PE transposes, projections =======
            # Load order: x, Wg-k block, then the kps/KT chain runs on PE
            # while Wg-v, Wg-q, Wo stream in behind it.  Wg-q row tiles are
            # also kept in natural layout (Wq) for the output-side fold
            # y = x @ (Wgq^T kv Wo^T): Q is never materialized.
            with tc.tile_pool(name="stg", bufs=3) as STG, \
                 tc.tile_pool(name="sbB", bufs=4) as SBB, \
                 tc.tile_pool(name="psT", bufs=2, space="PSUM") as PST, \
                 tc.tile_pool(name="psB", bufs=2, space="PSUM") as PSB, \
                 tc.tile_pool(name="psKT", bufs=2, space="PSUM") as PSKT, \
                 tc.tile_pool(name="psL", bufs=1, space="PSUM") as PSL:

                def load_dma(src, row0):
                    grp = []
                    for jj in range(4):
                        t = STG.tile([128, C], bf16, name=f"stg{jj}")
                        nc.gpsimd.dma_start(
                            out=t[:],
                            in_=src[row0 + jj * 128:row0 + (jj + 1) * 128, :],
                        )
                        grp.append(t)
                    return grp

                def load_tp(grp, dstT, col0):
                    for ci in range(CT):
                        ps = PST.tile([128, 512], bf16, name="pst")
                        for jj in range(4):
                            nc.tensor.transpose(
                                ps[:, ts(jj, 128)],
                                grp[jj][:, ts(ci, 128)],
                                ident_bf[:],
                            )
                        nc.any.tensor_copy(dstT[ci][:, ds(col0, 512)], ps[:])

                def load_group(src, row0, dstT, col0):
                    load_tp(load_dma(src, row0), dstT, col0)

                load_group(x, 0, xT, 0)
                load_group(Wg, C, WgT, C)  # k rows
                load_group(x, 512, xT, 512)
                load_small()
                # bias broadcast rows -> [128, C] tiles (one matmul each)
                bps = PSL.tile([128, C], f32, name="bps")
                mm(bps[:], lhsT=ones_bf[:], rhs=bgk_sb[:], start=True, stop=True)
                nc.any.tensor_copy(bgk_b[:], bps[:])
                bps2 = PSL.tile([128, C], f32, name="bps")
                mm(bps2[:], lhsT=ones_bf[:], rhs=bo_sb[:], start=True, stop=True)
                nc.any.tensor_copy(bo_b[:], bps2[:])

                for tj in range(NCH):
                    if tj == 4:
                        load_group(x, 2 * 512, xT, 2 * 512)
                    elif tj == 6:
                        load_group(x, 3 * 512, xT, 3 * 512)
                    kps = PSB.tile([128, C], f32, name="kps")
                    for ci in range(CT):
                        mm(kps[:], lhsT=xT[ci][:, ts(tj, 128)],
                           rhs=WgT[ci][:, ds(C, C)],
                           start=(ci == 0), stop=(ci == 3))
                    # Kn = kps + bgk (broadcast tile); n2 = sum Kn^2
                    nc.vector.tensor_tensor(
                        Kn[tj][:], kps[:], bgk_b[:], OP.add
                    )
                    junk = SBB.tile([128, C], f32, name="junk")
                    nc.vector.scalar_tensor_tensor(
                        junk[:], Kn[tj][:], 1.0, Kn[tj][:], OP.mult, OP.mult,
                        accum_out=n2_all[:, tj:tj + 1],
                    )
                    if tj % 4 == 3:
                        for ci in range(CT):
                            ps = PSKT.tile([128, 512], bf16, name="pskt")
                            for jj in range(4):
                                nc.tensor.transpose(
                                    ps[:, ts(jj, 128)],
                                    Kn[tj - 3 + jj][:, ts(ci, 128)],
                                    ident_bf[:],
                                )
                            nc.any.tensor_copy(
                                KTr[ci][:, ds((tj - 3) * 128, 512)], ps[:]
                            )
                    if tj == 3:
                        load_group(Wg, 2 * C, WgT, 2 * C)  # v rows
                    elif tj == 7:
                        for jj in range(4):  # q rows, natural layout only
                            nc.gpsimd.dma_start(
                                out=Wq[jj][:], in_=Wg[jj * 128:(jj + 1) * 128, :]
                            )
                    elif tj == 11:
                        load_group(Wo, 0, WoT, 0)

                # row scalings
                nc.vector.reciprocal(rn2[:], n2_all[:])
                nc.vector.tensor_scalar_mul(rn2n[:], rn2[:], -1.0)
                nc.scalar.activation(rn_all[:], rn2[:], AF.Sqrt)

                # lr row: lrT[1, T] then scatter to lrn [128, NCH]
                for tg in range(4):
                    lps = PSL.tile([1, 512], f32, name="lps")
                    for ci in range(CT):
                        mm(lps[:], lhsT=WlrT_sb[:, ci:ci + 1],
                           rhs=xT[ci][:, ds(tg * 512, 512)],
                           start=(ci == 0), stop=(ci == 3))
                    nc.scalar.activation(
                        lrT[:, ds(tg * 512, 512)], lps[:], AF.Identity,
                        bias=blr_sb[:, 0:1], scale=1.0,
                    )
                for i in range(NCH):
                    nc.gpsimd.dma_start(
                        out=lrn[:, i:i + 1], in_=lrT[0:1, ts(i, 128)]
                    )

            kcut = os.environ.get("KCUT", "")
            if kcut == "B":
                with tc.tile_pool(name="sbX", bufs=2) as SBX:
                    for tj in range(NCH):
                        zt = SBX.tile([128, C], f32, name="zt")
                        nc.vector.tensor_copy(zt[:], Kn[tj][:])
                        nc.sync.dma_start(out=out[ts(tj, 128), :], in_=zt[:])

            # ================= phase C: delta-rule recurrence ================
            # Pool scoping: construction pools (c2/c1/c1t) close after pair
            # NP-2 (all Mu/GX are built one pair ahead), freeing their PSUM
            # banks for the kv accumulators, which run during pair NP-1's
            # stalls.  PSG then hands its banks to psD for the output chain.
            if kcut in ("B",):
                pass
            elif True:
              with tc.tile_pool(name="sbC", bufs=4) as SBC, \
                 tc.tile_pool(name="sbR", bufs=4) as SBR, \
                 tc.tile_pool(name="sbD", bufs=4) as SBD:
                import contextlib
                _psg = contextlib.ExitStack()
                PSG = _psg.enter_context(
                    tc.tile_pool(name="psBIG", bufs=4, space="PSUM")
                )
                for ci in range(CT):
                    nc.gpsimd.memset(S_f32[ci][:], 0.0)

                def emit_V(i):
                    vps = PSG.tile([128, C], f32, name="big")
                    for ci in range(CT):
                        mm(vps[:], lhsT=xT[ci][:, ts(i, 128)],
                           rhs=WgT[ci][:, ds(2 * C, C)],
                           start=(ci == 0), stop=False)
                    mm(vps[:], lhsT=ones_bf[:], rhs=bgv_sb[:],
                       start=False, stop=True)
                    return vps

                def emit_P_S(i, close):
                    # P = Kraw_i S0 (+ cross term appended later for odd i)
                    pps = PSG.tile([128, C], f32, name="big")
                    for ci in range(CT):
                        mm(pps[:], lhsT=KTr[ci][:, ts(i, 128)], rhs=S_sb[ci][:],
                           start=(ci == 0), stop=(close and ci == 3))
                    return pps

                def emit_combine(i, vps, pps):
                    # rhs = rn * V  +  (-rn2) * P  (both legs on DVE so the
                    # chain has no cross-engine hop)
                    e1 = SBR.tile([128, C], bf16, name="e1")
                    nc.scalar.activation(
                        e1[:], vps[:], AF.Identity, scale=rn_all[:, i:i + 1]
                    )
                    if pps is None:
                        return e1
                    rhs = SBR.tile([128, C], bf16, name="rhs")
                    nc.vector.scalar_tensor_tensor(
                        rhs[:], pps[:], rn2n[:, i:i + 1], e1[:],
                        OP.mult, OP.add,
                    )
                    return rhs

                def emit_W(i, rhs):
                    wps = PSG.tile([128, C], f32, name="big")
                    mm(wps[:], lhsT=Mu[i][:], rhs=rhs[:], start=True, stop=True)
                    nc.vector.tensor_tensor(Wt[i][:], wps[:], rhs[:], OP.add)
                    nc.vector.tensor_scalar_mul(lru[i][:], Wt[i][:], lrn[:, i:i + 1])

                def emit_Supd(j):
                    a, b = 2 * j, 2 * j + 1
                    for ci in range(CT):
                        sd = PSG.tile([128, C], f32, name="big")
                        mm(sd[:], lhsT=Kn[a][:, ts(ci, 128)], rhs=Wt[a][:],
                           start=True, stop=False)
                        mm(sd[:], lhsT=Kn[b][:, ts(ci, 128)], rhs=Wt[b][:],
                           start=False, stop=True)
                        nc.vector.tensor_tensor(
                            S_f32[ci][:], sd[:], S_f32[ci][:], OP.add
                        )
                        nc.scalar.activation(S_sb[ci][:], S_f32[ci][:], AF.Identity)

                def do_pair(j, fill=None):
                    a, b = 2 * j, 2 * j + 1
                    vps_a = emit_V(a)
                    pps_a = emit_P_S(a, close=True) if j else None
                    vps_b = emit_V(b)
                    nBl = emit_gram(j + 1) if j < NP - 1 else None
                    if fill:
                        fill(0)
                    rhs_a = emit_combine(a, vps_a, pps_a)
                    emit_W(a, rhs_a)
                    pps_b = emit_P_S(b, close=False) if j else None
                    if j < NP - 1:
                        emit_constr(2 * j + 2, nBl[0])
                    if fill:
                        fill(1)
                    # cross term: P_b += gx^T W_a (closes / forms P_b group)
                    if pps_b is None:
                        pps_b = PSG.tile([128, C], f32, name="big")
                        mm(pps_b[:], lhsT=GX[j][:], rhs=Wt[a][:],
                           start=True, stop=True)
                    else:
                        mm(pps_b[:], lhsT=GX[j][:], rhs=Wt[a][:],
                           start=False, stop=True)
                    rhs_b = emit_combine(b, vps_b, pps_b)
                    emit_W(b, rhs_b)
                    if j < NP - 1:
                        emit_constr(2 * j + 3, nBl[1])
                    if fill:
                        fill(2)
                    if j < NP - 1:
                        emit_Supd(j)

                with tc.tile_pool(name="psC2", bufs=2, space="PSUM") as PSC2, \
                     tc.tile_pool(name="psC1", bufs=1, space="PSUM") as PSC1, \
                     tc.tile_pool(name="psCT", bufs=1, space="PSUM") as PSCT:

                    def emit_gram(j):
                        # pair grams: GA = [G_aa | gx], GB = G_bb; a=2j
                        a, b = 2 * j, 2 * j + 1
                        ga = PSC2.tile([128, 384], f32, name="c2")
                        for ci in range(CT):
                            mm(ga[:, 0:256], lhsT=KTr[ci][:, ts(a, 128)],
                               rhs=KTr[ci][:, ds(a * 128, 256)],
                               start=(ci == 0), stop=(ci == 3))
                        gb = PSC1.tile([128, 128], f32, name="c1")
                        for ci in range(CT):
                            mm(gb[:], lhsT=KTr[ci][:, ts(b, 128)],
                               rhs=KTr[ci][:, ts(b, 128)],
                               start=(ci == 0), stop=(ci == 3))
                        nc.scalar.activation(GX[j][:], ga[:, 128:256], AF.Identity)
                        # B = -tril(G,-1) * rn2 (rows): one fused DVE op each
                        Bl_a = SBC.tile([128, 128], bf16, name="Bla")
                        nc.vector.scalar_tensor_tensor(
                            Bl_a[:], ga[:, 0:128], rn2[:, a:a + 1], maskLn[:],
                            OP.mult, OP.mult,
                        )
                        Bl_b = SBC.tile([128, 128], bf16, name="Blb")
                        nc.vector.scalar_tensor_tensor(
                            Bl_b[:], gb[:], rn2[:, b:b + 1], maskLn[:],
                            OP.mult, OP.mult,
                        )
                        return Bl_a, Bl_b

                    def emit_constr(i, Bl):
                        # Mu[i] = (F1 + F2 + F1@F2)^T, F1 = B+B2+B3,
                        # F2 = B4+B8 (exact Neumann through degree 11).
                        # T1 = [Bu | B2u | B2l] lets [B3u | B4u] come from a
                        # single free-256 matmul with stationary B2l.
                        tps = PSCT.tile([128, 128], bf16, name="c1t")
                        nc.tensor.transpose(tps[:], Bl[:], ident_bf[:])
                        T1 = SBC.tile([128, 384], bf16, name="T1")
                        nc.scalar.activation(T1[:, 0:128], tps[:], AF.Identity)
                        Bu = T1[:, 0:128]

                        p1 = PSC2.tile([128, 384], f32, name="c2")
                        mm(p1[:, 0:128], lhsT=Bl[:], rhs=Bu, start=True, stop=True)
                        mm(p1[:, 128:256], lhsT=Bu, rhs=Bl[:], start=True, stop=True)
                        nc.scalar.activation(T1[:, 128:384], p1[:, 0:256], AF.Identity)
                        B2u, B2l = T1[:, 128:256], T1[:, 256:384]

                        pX = PSC2.tile([128, 384], f32, name="c2")
                        mm(pX[:, 0:256], lhsT=B2l, rhs=T1[:, 0:256],
                           start=True, stop=True)  # [B3u | B4u]
                        mm(pX[:, 256:384], lhsT=B2u, rhs=B2l, start=True, stop=True)
                        X = SBC.tile([128, 384], bf16, name="X")
                        nc.scalar.activation(X[:], pX[:], AF.Identity)
                        B3u, B4u, B4l = X[:, 0:128], X[:, 128:256], X[:, 256:384]

                        p3 = PSC2.tile([128, 384], f32, name="c2")
                        mm(p3[:, 0:128], lhsT=B4l, rhs=B4u, start=True, stop=True)
                        mm(p3[:, 128:256], lhsT=B4u, rhs=B4l, start=True, stop=True)
                        B8 = SBC.tile([128, 256], bf16, name="B8")
                        nc.scalar.activation(B8[:], p3[:, 0:256], AF.Identity)

                        # F1u = Bu + B2u + B3u; mp = (F1 F2)^T = F2u @ F1u;
                        # Mu = mp + F1u + F2u (adds done on DVE, not PE)
                        bb2 = SBC.tile([128, 128], bf16, name="bb2")
                        nc.vector.tensor_tensor(bb2[:], Bu, B2u, OP.add)
                        F1u = SBC.tile([128, 128], bf16, name="F1u")
                        nc.vector.tensor_tensor(F1u[:], B3u, bb2[:], OP.add)

                        F2 = SBC.tile([128, 256], bf16, name="F2")
                        nc.vector.tensor_tensor(F2[:], B8[:], X[:, 128:384], OP.add)
                        F2u, F2l = F2[:, 0:128], F2[:, 128:256]
                        fsum = SBC.tile([128, 128], bf16, name="fsum")
                        nc.vector.tensor_tensor(fsum[:], F1u[:], F2u, OP.add)

                        mp = PSC1.tile([128, 128], f32, name="c1")
                        mm(mp[:], lhsT=F2l, rhs=F1u[:], start=True, stop=True)
                        nc.vector.tensor_tensor(Mu[i][:], mp[:], fsum[:], OP.add)

                    # prologue: pair-0 construction
                    Bl_a, Bl_b = emit_gram(0)
                    emit_constr(0, Bl_a)
                    emit_constr(1, Bl_b)
                    for j in range(NP - 1):
                        do_pair(j)

                if kcut == "C":
                    do_pair(NP - 1)
                    _psg.close()
                    with tc.tile_pool(name="sbX", bufs=2) as SBX:
                        for tj in range(NCH):
                            zt = SBX.tile([128, C], f32, name="zt")
                            nc.vector.tensor_copy(zt[:], Wt[tj][:])
                            nc.sync.dma_start(out=out[ts(tj, 128), :], in_=zt[:])
                else:
                    # ============== phase D: outputs =========================
                    with tc.tile_pool(name="psKV", bufs=1, space="PSUM") as PSKV:
                        kvt_ps = [PSKV.tile([128, C], f32, name=f"kvt{vi}")
                                  for vi in range(CT)]

                        def emit_kv(i0, i1):
                            for i in range(i0, i1):
                                for vi in range(CT):
                                    mm(kvt_ps[vi][:],
                                       lhsT=lru[i][:, ts(vi, 128)],
                                       rhs=Kn[i][:],
                                       start=(i == 0), stop=(i == NCH - 1))

                        emit_kv(0, 11)
                        do_pair(NP - 1, fill=lambda s: emit_kv(*[(11, 12), (12, 13),
                                                                 (13, 14)][s]))
                        emit_kv(14, NCH)
                        for vi in range(CT):
                            nc.any.tensor_copy(KVT[vi][:], kvt_ps[vi][:])

                    _psg.close()
                    with tc.tile_pool(name="psD", bufs=2, space="PSUM") as PSD, \
                         tc.tile_pool(name="psDq", bufs=1, space="PSUM") as PSDQ:
                        for ki in range(CT):
                            kwp = PSD.tile([128, C], f32, name="dps")
                            for vi in range(CT):
                                mm(kwp[:], lhsT=KVT[vi][:, ts(ki, 128)],
                                   rhs=WoT[vi][:],
                                   start=(vi == 0), stop=(vi == 3))
                            nc.any.tensor_copy(KVW[ki][:], kwp[:])

                        # fold Q into the output: R = Wgq^T @ KVW,
                        # brow = bgq @ KVW + bo, out = x @ R + brow.
                        bqp = PSDQ.tile([1, C], f32, name="bqp")
                        for ri in range(CT):
                            mm(bqp[:], lhsT=bgq_sb[:, ri:ri + 1], rhs=KVW[ri][:],
                               start=(ri == 0), stop=(ri == 3))
                        brow = SBD.tile([1, C], bf16, name="brow")
                        nc.vector.tensor_tensor(brow[:], bqp[:], bo_sb[:], OP.add)
                        for ci in range(CT):
                            rps = PSD.tile([128, C], f32, name="dps")
                            for ri in range(CT):
                                mm(rps[:], lhsT=Wq[ri][:, ts(ci, 128)],
                                   rhs=KVW[ri][:],
                                   start=(ri == 0), stop=(ri == 3))
                            nc.any.tensor_copy(Rt[ci][:], rps[:])
                        b2p = PSD.tile([128, C], f32, name="dps")
                        mm(b2p[:], lhsT=ones_bf[:], rhs=brow[:], start=True, stop=True)
                        nc.any.tensor_copy(bo2_b[:], b2p[:])

                        for tj in range(NCH):
                            zps = PSD.tile([128, C], f32, name="dps")
                            for ci in range(CT):
                                mm(zps[:], lhsT=xT[ci][:, ts(tj, 128)],
                                   rhs=Rt[ci][:],
                                   start=(ci == 0), stop=(ci == 3))
                            zt = SBD.tile([128, C], f32, name="zt")
                            nc.vector.tensor_tensor(zt[:], zps[:], bo2_b[:], OP.add)
                            nc.sync.dma_start(out=out[ts(tj, 128), :], in_=zt[:])

                    if dbg:
                        for i in range(NCH):
                            dw = SBD.tile([128, C], f32, name="du")
                            nc.vector.tensor_copy(dw[:], Wt[i][:])
                            nc.sync.dma_start(out=dbg_W[ts(i, 128), :], in_=dw[:])
                            dk = SBD.tile([128, C], f32, name="du")
                            nc.vector.tensor_copy(dk[:], Kn[i][:])
                            nc.sync.dma_start(out=dbg_K[ts(i, 128), :], in_=dk[:])
                        for ci in range(CT):
                            dsb = SBD.tile([128, C], f32, name="du")
                            nc.vector.tensor_copy(dsb[:], S_f32[ci][:])
                            nc.sync.dma_start(out=dbg_S[ts(ci, 128), :], in_=dsb[:])
                            dkv = SBD.tile([128, C], f32, name="du")
                            nc.vector.tensor_copy(dkv[:], KVT[ci][:])
                            nc.sync.dma_start(out=dbg_KVT[ts(ci, 128), :], in_=dkv[:])

    nc.finalize()
    return nc


def _get_nc():
    if "nc" not in _CACHE:
        _CACHE["nc"] = _build()
    return _CACHE["nc"]


def _in_maps(inputs):
    def f(a):
        return np.ascontiguousarray(np.asarray(a, dtype=np.float32))

    x = f(inputs["x"])
    shared = {k: f(inputs[k]) for k in ("Wg", "bg", "Wlr", "blr", "Wo", "bo")}
    return [{"x": x[i], **shared} for i in range(N)]


def _run(in_maps, **kw):
    from concourse.bass_utils import run_bass_kernel_spmd

    nc = _get_nc()
    return run_bass_kernel_spmd(nc, in_maps, list(range(N)), **kw)


def kernel(**inputs) -> np.ndarray:
    res = _run(_in_maps(inputs))
    return np.stack([res.results[i]["out"] for i in range(N)]).astype(np.float32)


# revision 45
# speedup vs baseline: 1.0540x; 1.0016x over previous
# Bass/Trainium2 kernel for nn_Delta (DeltaNet-style recurrence).
#
# Problem (hardcoded): N=8, T=2048, C=512, fp32 I/O.
#   g = x @ Wg.T + bg ; q,k,v = split(g) ; lr = x @ Wlr.T + blr
#   khat = k / ||k||
#   delta-rule scan:  u_t = v_t - khat_t @ S ; S += outer(khat_t, u_t)
#   kv = sum_t khat_t (x) (lr_t * u_t) ; y = q @ kv ; out = y @ Wo.T + bo
#
# Sharding: data-parallel over N across the 8 cores (sample i -> core i),
# weights replicated. No collectives.
#
# Per-core algorithm: chunked parallel delta rule, chunk L=128, in
# "W-space": with D = diag(1/||k_raw||) per chunk and W = D^-1-free
# substitution W = D u, the recurrence becomes
#   (I + D^2 G) W = D V - D^2 (Kraw S + cross),   G = tril(Kraw Kraw^T, -1)
#   S += Kraw^T W,   kv = Kraw^T (lr . W)
# so only RAW projections appear in matmuls; the 1/||k|| and 1/||k||^2
# row scalings ride on PSUM evacuations (per-partition scale APs).
#
# (I + B)^-1 with B = -D^2 G is truncated exactly through degree 11 via
# the two-factor form (I+F1)(I+F2), F1 = B+B^2+B^3, F2 = B^4+B^8,
# applied merged: W = rhs + M' rhs with M' = F1+F2+F1F2  (measured
# truncation error 2.3e-3 in f64 on this data, far below the bf16 noise).
#
# Q is never materialized: y = q @ kv folds into out = x @ R + brow with
# R = Wgq^T (kv Wo^T) and brow = bgq (kv Wo^T) + bo, reusing resident xT.
#
# No DMA transposes: x, Wg, Wo are cast-DMA'd naturally and transposed
# on the tensor engine (transpose mode), as is Bl -> Bu.

import os

import numpy as np

N, T, C = 8, 2048, 512
L = 128
NCH = T // L  # 16 chunks
NP = NCH // 2  # 8 chunk pairs
CT = C // 128  # 4 c-tiles

_CACHE = {}


def _build():
    import concourse.bacc as bacc
    import concourse.mybir as mybir
    import concourse.tile as tile
    from concourse.bass import ts, ds
    from concourse.masks import make_identity, make_lower_triangular

    f32 = mybir.dt.float32
    bf16 = mybir.dt.bfloat16
    AF = mybir.ActivationFunctionType
    OP = mybir.AluOpType

    nc = bacc.Bacc("TRN2")
    x = nc.declare_dram_parameter("x", [T, C], f32, isOutput=False)
    Wg = nc.declare_dram_parameter("Wg", [3 * C, C], f32, isOutput=False)
    bg = nc.declare_dram_parameter("bg", [3 * C], f32, isOutput=False)
    Wlr = nc.declare_dram_parameter("Wlr", [1, C], f32, isOutput=False)
    blr = nc.declare_dram_parameter("blr", [1], f32, isOutput=False)
    Wo = nc.declare_dram_parameter("Wo", [C, C], f32, isOutput=False)
    bo = nc.declare_dram_parameter("bo", [C], f32, isOutput=False)
    out = nc.declare_dram_parameter("out", [T, C], f32, isOutput=True)
    dbg = os.environ.get("KDBG") == "1"
    if dbg:
        dbg_W = nc.declare_dram_parameter("dbg_W", [T, C], f32, isOutput=True)
        dbg_K = nc.declare_dram_parameter("dbg_K", [T, C], f32, isOutput=True)
        dbg_S = nc.declare_dram_parameter("dbg_S", [C, C], f32, isOutput=True)
        dbg_KVT = nc.declare_dram_parameter("dbg_KVT", [C, C], f32, isOutput=True)

    mm = nc.tensor.matmul

    with tile.TileContext(nc) as tc:
        with tc.tile_pool(name="persist", bufs=1) as P:
            # ---- constants / small tensors ----
            maskLn = P.tile([128, 128], f32, name="maskLn")
            make_lower_triangular(nc, maskLn[:], val=-1.0, diag=False)
            ones_bf = P.tile([1, 128], bf16, name="ones_bf")
            nc.vector.memset(ones_bf[:], 1.0)
            ident_bf = P.tile([128, 128], bf16, name="ident_bf")
            make_identity(nc, ident_bf[:])

            bgk_sb = P.tile([1, C], bf16, name="bgk_sb")
            bgv_sb = P.tile([1, C], bf16, name="bgv_sb")
            bo_sb = P.tile([1, C], bf16, name="bo_sb")
            bgq_sb = P.tile([128, CT], bf16, name="bgq_sb")
            WlrT_sb = P.tile([128, CT], bf16, name="WlrT_sb")
            blr_sb = P.tile([1, 1], f32, name="blr_sb")

            def load_small():
                nc.gpsimd.dma_start(out=bgk_sb[:], in_=bg[C:2 * C])
                nc.gpsimd.dma_start(out=bgv_sb[:], in_=bg[2 * C:3 * C])
                nc.gpsimd.dma_start(out=bo_sb[:], in_=bo[:])
                nc.gpsimd.dma_start(
                    out=bgq_sb[:], in_=bg[0:C].rearrange("(i p) -> p i", p=128)
                )
                nc.gpsimd.dma_start(
                    out=WlrT_sb[:], in_=Wlr[0, :].rearrange("(i p) -> p i", p=128)
                )
                nc.gpsimd.dma_start(out=blr_sb[:], in_=blr[:])

            # ---- persistent tensors ----
            xT = [P.tile([128, T], bf16, name=f"xT{i}") for i in range(CT)]
            WgT = [P.tile([128, 3 * C], bf16, name=f"WgT{i}") for i in range(CT)]
            WoT = [P.tile([128, C], bf16, name=f"WoT{i}") for i in range(CT)]
            KTr = [P.tile([128, T], bf16, name=f"KTr{i}") for i in range(CT)]
            Kn = [P.tile([128, C], bf16, name=f"Kn{i}") for i in range(NCH)]
            Wt = [P.tile([128, C], bf16, name=f"Wt{i}") for i in range(NCH)]
            lru = [P.tile([128, C], bf16, name=f"lru{i}") for i in range(NCH)]
            Wq = [P.tile([128, C], bf16, name=f"Wq{i}") for i in range(CT)]
            Rt = [P.tile([128, C], bf16, name=f"Rt{i}") for i in range(CT)]
            bo2_b = P.tile([128, C], f32, name="bo2_b")
            S_f32 = [P.tile([128, C], f32, name=f"Sf{i}") for i in range(CT)]
            S_sb = [P.tile([128, C], bf16, name=f"S{i}") for i in range(CT)]
            KVT = [P.tile([128, C], bf16, name=f"KVT{i}") for i in range(CT)]
            KVW = [P.tile([128, C], bf16, name=f"KVW{i}") for i in range(CT)]
            bgk_b = P.tile([128, C], bf16, name="bgk_b")
            bo_b = P.tile([128, C], f32, name="bo_b")
            lrT = P.tile([1, T], f32, name="lrT")
            lrn = P.tile([128, NCH], f32, name="lrn")
            n2_all = P.tile([128, NCH], f32, name="n2_all")
            rn2 = P.tile([128, NCH], f32, name="rn2")  # 1/||k||^2
            rn2n = P.tile([128, NCH], f32, name="rn2n")  # -1/||k||^2
            rn_all = P.tile([128, NCH], f32, name="rn_all")  # 1/||k||
            # per-chunk construction outputs (consumed next pair at latest)
            Mu = [P.tile([128, 128], bf16, name=f"Mu{i}") for i in range(NCH)]
            GX = [P.tile([128, 128], bf16, name=f"GX{i}") for i in range(NP)]

            # ============ phase A+B: loads, PE transposes, projections =======
            # Load order: x, Wg-k block, then the kps/KT chain runs on PE
            # while Wg-v, Wg-q, Wo stream in behind it.  Wg-q row tiles are
            # also kept in natural layout (Wq) for the output-side fold
            # y = x @ (Wgq^T kv Wo^T): Q is never materialized.
            with tc.tile_pool(name="stg", bufs=3) as STG, \
                 tc.tile_pool(name="sbB", bufs=4) as SBB, \
                 tc.tile_pool(name="psT", bufs=2, space="PSUM") as PST, \
                 tc.tile_pool(name="psB", bufs=2, space="PSUM") as PSB, \
                 tc.tile_pool(name="psKT", bufs=2, space="PSUM") as PSKT, \
                 tc.tile_pool(name="psL", bufs=1, space="PSUM") as PSL:

                def load_dma(src, row0):
                    grp = []
                    for jj in range(4):
                        t = STG.tile([128, C], bf16, name=f"stg{jj}")
                        nc.gpsimd.dma_start(
                            out=t[:],
                            in_=src[row0 + jj * 128:row0 + (jj + 1) * 128, :],
                        )
                        grp.append(t)
                    return grp

                def load_tp(grp, dstT, col0):
                    for ci in range(CT):
                        ps = PST.tile([128, 512], bf16, name="pst")
                        for jj in range(4):
                            nc.tensor.transpose(
                                ps[:, ts(jj, 128)],
                                grp[jj][:, ts(ci, 128)],
                                ident_bf[:],
                            )
                        nc.any.tensor_copy(dstT[ci][:, ds(col0, 512)], ps[:])

                def load_group(src, row0, dstT, col0):
                    load_tp(load_dma(src, row0), dstT, col0)

                load_group(x, 0, xT, 0)
                load_group(Wg, C, WgT, C)  # k rows
                load_group(x, 512, xT, 512)
                load_small()
                # bias broadcast rows -> [128, C] tiles (one matmul each)
                bps = PSL.tile([128, C], f32, name="bps")
                mm(bps[:], lhsT=ones_bf[:], rhs=bgk_sb[:], start=True, stop=True)
                nc.any.tensor_copy(bgk_b[:], bps[:])
                bps2 = PSL.tile([128, C], f32, name="bps")
                mm(bps2[:], lhsT=ones_bf[:], rhs=bo_sb[:], start=True, stop=True)
                nc.any.tensor_copy(bo_b[:], bps2[:])

                for tj in range(NCH):
                    if tj == 4:
                        load_group(x, 2 * 512, xT, 2 * 512)
                    elif tj == 6:
                        load_group(x, 3 * 512, xT, 3 * 512)
                    kps = PSB.tile([128, C], f32, name="kps")
                    for ci in range(CT):
                        mm(kps[:], lhsT=xT[ci][:, ts(tj, 128)],
                           rhs=WgT[ci][:, ds(C, C)],
                           start=(ci == 0), stop=(ci == 3))
                    # Kn = kps + bgk (broadcast tile); n2 = sum Kn^2
                    nc.vector.tensor_tensor(
                        Kn[tj][:], kps[:], bgk_b[:], OP.add
                    )
                    junk = SBB.tile([128, C], f32, name="junk")
                    nc.vector.scalar_tensor_tensor(
                        junk[:], Kn[tj][:], 1.0, Kn[tj][:], OP.mult, OP.mult,
                        accum_out=n2_all[:, tj:tj + 1],
                    )
                    if tj % 4 == 3:
                        for ci in range(CT):
                            ps = PSKT.tile([128, 512], bf16, name="pskt")
                            for jj in range(4):
                                nc.tensor.transpose(
                                    ps[:, ts(jj, 128)],
                                    Kn[tj - 3 + jj][:, ts(ci, 128)],
                                    ident_bf[:],
                                )
                            nc.any.tensor_copy(
                                KTr[ci][:, ds((tj - 3) * 128, 512)], ps[:]
                            )
                    if tj == 3:
                        load_group(Wg, 2 * C, WgT, 2 * C)  # v rows
                    elif tj == 7:
                        for jj in range(4):  # q rows, natural layout only
                            nc.gpsimd.dma_start(
                                out=Wq[jj][:], in_=Wg[jj * 128:(jj + 1) * 128, :]
                            )
                    elif tj == 11:
                        load_group(Wo, 0, WoT, 0)

                # row scalings
                nc.vector.reciprocal(rn2[:], n2_all[:])
                nc.vector.tensor_scalar_mul(rn2n[:], rn2[:], -1.0)
                nc.scalar.activation(rn_all[:], rn2[:], AF.Sqrt)

                # lr row: lrT[1, T] then scatter to lrn [128, NCH]
                for tg in range(4):
                    lps = PSL.tile([1, 512], f32, name="lps")
                    for ci in range(CT):
                        mm(lps[:], lhsT=WlrT_sb[:, ci:ci + 1],
                           rhs=xT[ci][:, ds(tg * 512, 512)],
                           start=(ci == 0), stop=(ci == 3))
                    nc.scalar.activation(
                        lrT[:, ds(tg * 512, 512)], lps[:], AF.Identity,
                        bias=blr_sb[:, 0:1], scale=1.0,
                    )
                for i in range(NCH):
                    nc.gpsimd.dma_start(
                        out=lrn[:, i:i + 1], in_=lrT[0:1, ts(i, 128)]
                    )

            kcut = os.environ.get("KCUT", "")
            if kcut == "B":
                with tc.tile_pool(name="sbX", bufs=2) as SBX:
                    for tj in range(NCH):
                        zt = SBX.tile([128, C], f32, name="zt")
                        nc.vector.tensor_copy(zt[:], Kn[tj][:])
                        nc.sync.dma_start(out=out[ts(tj, 128), :], in_=zt[:])

            # ================= phase C: delta-rule recurrence ================
            # Pool scoping: construction pools (c2/c1/c1t) close after pair
            # NP-2 (all Mu/GX are built one pair ahead), freeing their PSUM
            # banks for the kv accumulators, which run during pair NP-1's
            # stalls.  PSG then hands its banks to psD for the output chain.
            if kcut in ("B",):
                pass
            elif True:
              with tc.tile_pool(name="sbC", bufs=4) as SBC, \
                 tc.tile_pool(name="sbR", bufs=4) as SBR, \
                 tc.tile_pool(name="sbD", bufs=4) as SBD:
                import contextlib
                _psg = contextlib.ExitStack()
                PSG = _psg.enter_context(
                    tc.tile_pool(name="psBIG", bufs=4, space="PSUM")
                )
                for ci in range(CT):
                    nc.gpsimd.memset(S_f32[ci][:], 0.0)

                def emit_V(i):
                    vps = PSG.tile([128, C], f32, name="big")
                    for ci in range(CT):
                        mm(vps[:], lhsT=xT[ci][:, ts(i, 128)],
                           rhs=WgT[ci][:, ds(2 * C, C)],
                           start=(ci == 0), stop=False)
                    mm(vps[:], lhsT=ones_bf[:], rhs=bgv_sb[:],
                       start=False, stop=True)
                    return vps

                def emit_P_S(i, close):
                    # P = Kraw_i S0 (+ cross term appended later for odd i)
                    pps = PSG.tile([128, C], f32, name="big")
                    for ci in range(CT):
                        mm(pps[:], lhsT=KTr[ci][:, ts(i, 128)], rhs=S_sb[ci][:],
                           start=(ci == 0), stop=(close and ci == 3))
                    return pps

                def emit_combine(i, vps, pps):
                    # rhs = rn * V  +  (-rn2) * P  (both legs on DVE so the
                    # chain has no cross-engine hop)
                    e1 = SBR.tile([128, C], bf16, name="e1")
                    nc.scalar.activation(
                        e1[:], vps[:], AF.Identity, scale=rn_all[:, i:i + 1]
                    )
                    if pps is None:
                        return e1
                    rhs = SBR.tile([128, C], bf16, name="rhs")
                    nc.vector.scalar_tensor_tensor(
                        rhs[:], pps[:], rn2n[:, i:i + 1], e1[:],
                        OP.mult, OP.add,
                    )
                    return rhs

                def emit_W(i, rhs):
                    wps = PSG.tile([128, C], f32, name="big")
                    mm(wps[:], lhsT=Mu[i][:], rhs=rhs[:], start=True, stop=True)
                    nc.vector.tensor_tensor(Wt[i][:], wps[:], rhs[:], OP.add)
                    nc.vector.tensor_scalar_mul(lru[i][:], Wt[i][:], lrn[:, i:i + 1])

                def emit_Supd(j):
                    a, b = 2 * j, 2 * j + 1
                    for ci in range(CT):
                        sd = PSG.tile([128, C], f32, name="big")
                        mm(sd[:], lhsT=Kn[a][:, ts(ci, 128)], rhs=Wt[a][:],
                           start=True, stop=False)
                        mm(sd[:], lhsT=Kn[b][:, ts(ci, 128)], rhs=Wt[b][:],
                           start=False, stop=True)
                        nc.vector.tensor_tensor(
                            S_f32[ci][:], sd[:], S_f32[ci][:], OP.add
                        )
                        nc.scalar.activation(S_sb[ci][:], S_f32[ci][:], AF.Identity)

                def do_pair(j, fill=None):
                    a, b = 2 * j, 2 * j + 1
                    vps_a = emit_V(a)
                    pps_a = emit_P_S(a, close=True) if j else None
                    vps_b = emit_V(b)
                    if fill:
                        fill(0)
                    rhs_a = emit_combine(a, vps_a, pps_a)
                    emit_W(a, rhs_a)
                    pps_b = emit_P_S(b, close=False) if j else None
                    nBl = emit_gram(j + 1) if j < NP - 1 else None
                    if j < NP - 1:
                        emit_constr(2 * j + 2, nBl[0])
                    if fill:
                        fill(1)
                    # cross term: P_b += gx^T W_a (closes / forms P_b group)
                    if pps_b is None:
                        pps_b = PSG.tile([128, C], f32, name="big")
                        mm(pps_b[:], lhsT=GX[j][:], rhs=Wt[a][:],
                           start=True, stop=True)
                    else:
                        mm(pps_b[:], lhsT=GX[j][:], rhs=Wt[a][:],
                           start=False, stop=True)
                    rhs_b = emit_combine(b, vps_b, pps_b)
                    emit_W(b, rhs_b)
                    if j < NP - 1:
                        emit_constr(2 * j + 3, nBl[1])
                    if fill:
                        fill(2)
                    if j < NP - 1:
                        emit_Supd(j)

                with tc.tile_pool(name="psC2", bufs=2, space="PSUM") as PSC2, \
                     tc.tile_pool(name="psC1", bufs=1, space="PSUM") as PSC1, \
                     tc.tile_pool(name="psCT", bufs=1, space="PSUM") as PSCT:

                    def emit_gram(j):
                        # pair grams: GA = [G_aa | gx], GB = G_bb; a=2j
                        a, b = 2 * j, 2 * j + 1
                        ga = PSC2.tile([128, 384], f32, name="c2")
                        for ci in range(CT):
                            mm(ga[:, 0:256], lhsT=KTr[ci][:, ts(a, 128)],
                               rhs=KTr[ci][:, ds(a * 128, 256)],
                               start=(ci == 0), stop=(ci == 3))
                        gb = PSC1.tile([128, 128], f32, name="c1")
                        for ci in range(CT):
                            mm(gb[:], lhsT=KTr[ci][:, ts(b, 128)],
                               rhs=KTr[ci][:, ts(b, 128)],
                               start=(ci == 0), stop=(ci == 3))
                        nc.scalar.activation(GX[j][:], ga[:, 128:256], AF.Identity)
                        # B = -tril(G,-1) * rn2 (rows): one fused DVE op each
                        Bl_a = SBC.tile([128, 128], bf16, name="Bla")
                        nc.vector.scalar_tensor_tensor(
                            Bl_a[:], ga[:, 0:128], rn2[:, a:a + 1], maskLn[:],
                            OP.mult, OP.mult,
                        )
                        Bl_b = SBC.tile([128, 128], bf16, name="Blb")
                        nc.vector.scalar_tensor_tensor(
                            Bl_b[:], gb[:], rn2[:, b:b + 1], maskLn[:],
                            OP.mult, OP.mult,
                        )
                        return Bl_a, Bl_b

                    def emit_constr(i, Bl):
                        # Mu[i] = (F1 + F2 + F1@F2)^T, F1 = B+B2+B3,
                        # F2 = B4+B8 (exact Neumann through degree 11).
                        # T1 = [Bu | B2u | B2l] lets [B3u | B4u] come from a
                        # single free-256 matmul with stationary B2l.
                        tps = PSCT.tile([128, 128], bf16, name="c1t")
                        nc.tensor.transpose(tps[:], Bl[:], ident_bf[:])
                        T1 = SBC.tile([128, 384], bf16, name="T1")
                        nc.scalar.activation(T1[:, 0:128], tps[:], AF.Identity)
                        Bu = T1[:, 0:128]

                        p1 = PSC2.tile([128, 384], f32, name="c2")
                        mm(p1[:, 0:128], lhsT=Bl[:], rhs=Bu, start=True, stop=True)
                        mm(p1[:, 128:256], lhsT=Bu, rhs=Bl[:], start=True, stop=True)
                        nc.scalar.activation(T1[:, 128:384], p1[:, 0:256], AF.Identity)
                        B2u, B2l = T1[:, 128:256], T1[:, 256:384]

                        pX = PSC2.tile([128, 384], f32, name="c2")
                        mm(pX[:, 0:256], lhsT=B2l, rhs=T1[:, 0:256],
                           start=True, stop=True)  # [B3u | B4u]
                        mm(pX[:, 256:384], lhsT=B2u, rhs=B2l, start=True, stop=True)
                        X = SBC.tile([128, 384], bf16, name="X")
                        nc.scalar.activation(X[:], pX[:], AF.Identity)
                        B3u, B4u, B4l = X[:, 0:128], X[:, 128:256], X[:, 256:384]

                        p3 = PSC2.tile([128, 384], f32, name="c2")
                        mm(p3[:, 0:128], lhsT=B4l, rhs=B4u, start=True, stop=True)
                        mm(p3[:, 128:256], lhsT=B4u, rhs=B4l, start=True, stop=True)
                        B8 = SBC.tile([128, 256], bf16, name="B8")
                        nc.scalar.activation(B8[:], p3[:, 0:256], AF.Identity)

                        # F1u = Bu + B2u + B3u; mp = (F1 F2)^T = F2u @ F1u;
                        # Mu = mp + F1u + F2u (adds done on DVE, not PE)
                        bb2 = SBC.tile([128, 128], bf16, name="bb2")
                        nc.vector.tensor_tensor(bb2[:], Bu, B2u, OP.add)
                        F1u = SBC.tile([128, 128], bf16, name="F1u")
                        nc.vector.tensor_tensor(F1u[:], B3u, bb2[:], OP.add)

                        F2 = SBC.tile([128, 256], bf16, name="F2")
                        nc.vector.tensor_tensor(F2[:], B8[:], X[:, 128:384], OP.add)
                        F2u, F2l = F2[:, 0:128], F2[:, 128:256]
                        fsum = SBC.tile([128, 128], bf16, name="fsum")
                        nc.vector.tensor_tensor(fsum[:], F1u[:], F2u, OP.add)

                        mp = PSC1.tile([128, 128], f32, name="c1")
                        mm(mp[:], lhsT=F2l, rhs=F1u[:], start=True, stop=True)
                        nc.vector.tensor_tensor(Mu[i][:], mp[:], fsum[:], OP.add)

                    # prologue: pair-0 construction
                    Bl_a, Bl_b = emit_gram(0)
                    emit_constr(0, Bl_a)
                    emit_constr(1, Bl_b)
                    for j in range(NP - 1):
                        do_pair(j)

                if kcut == "C":
                    do_pair(NP - 1)
                    _psg.close()
                    with tc.tile_pool(name="sbX", bufs=2) as SBX:
                        for tj in range(NCH):
                            zt = SBX.tile([128, C], f32, name="zt")
                            nc.vector.tensor_copy(zt[:], Wt[tj][:])
                            nc.sync.dma_start(out=out[ts(tj, 128), :], in_=zt[:])
                else:
                    # ============== phase D: outputs =========================
                    with tc.tile_pool(name="psKV", bufs=1, space="PSUM") as PSKV:
                        kvt_ps = [PSKV.tile([128, C], f32, name=f"kvt{vi}")
                                  for vi in range(CT)]

                        def emit_kv(i0, i1):
                            for i in range(i0, i1):
                                for vi in range(CT):
                                    mm(kvt_ps[vi][:],
                                       lhsT=lru[i][:, ts(vi, 128)],
                                       rhs=Kn[i][:],
                                       start=(i == 0), stop=(i == NCH - 1))

                        emit_kv(0, 11)
                        do_pair(NP - 1, fill=lambda s: emit_kv(*[(11, 12), (12, 13),
                                                                 (13, 14)][s]))
                        emit_kv(14, NCH)
                        for vi in range(CT):
                            nc.any.tensor_copy(KVT[vi][:], kvt_ps[vi][:])

                    _psg.close()
                    with tc.tile_pool(name="psD", bufs=2, space="PSUM") as PSD, \
                         tc.tile_pool(name="psDq", bufs=1, space="PSUM") as PSDQ:
                        for ki in range(CT):
                            kwp = PSD.tile([128, C], f32, name="dps")
                            for vi in range(CT):
                                mm(kwp[:], lhsT=KVT[vi][:, ts(ki, 128)],
                                   rhs=WoT[vi][:],
                                   start=(vi == 0), stop=(vi == 3))
                            nc.any.tensor_copy(KVW[ki][:], kwp[:])

                        # fold Q into the output: R = Wgq^T @ KVW,
                        # brow = bgq @ KVW + bo, out = x @ R + brow.
                        bqp = PSDQ.tile([1, C], f32, name="bqp")
                        for ri in range(CT):
                            mm(bqp[:], lhsT=bgq_sb[:, ri:ri + 1], rhs=KVW[ri][:],
                               start=(ri == 0), stop=(ri == 3))
                        brow = SBD.tile([1, C], bf16, name="brow")
                        nc.vector.tensor_tensor(brow[:], bqp[:], bo_sb[:], OP.add)
                        for ci in range(CT):
                            rps = PSD.tile([128, C], f32, name="dps")
                            for ri in range(CT):
                                mm(rps[:], lhsT=Wq[ri][:, ts(ci, 128)],
                                   rhs=KVW[ri][:],
                                   start=(ri == 0), stop=(ri == 3))
                            nc.any.tensor_copy(Rt[ci][:], rps[:])
                        b2p = PSD.tile([128, C], f32, name="dps")
                        mm(b2p[:], lhsT=ones_bf[:], rhs=brow[:], start=True, stop=True)
                        nc.any.tensor_copy(bo2_b[:], b2p[:])

                        for tj in range(NCH):
                            zps = PSD.tile([128, C], f32, name="dps")
                            for ci in range(CT):
                                mm(zps[:], lhsT=xT[ci][:, ts(tj, 128)],
                                   rhs=Rt[ci][:],
                                   start=(ci == 0), stop=(ci == 3))
                            zt = SBD.tile([128, C], f32, name="zt")
                            nc.vector.tensor_tensor(zt[:], zps[:], bo2_b[:], OP.add)
                            nc.sync.dma_start(out=out[ts(tj, 128), :], in_=zt[:])

                    if dbg:
                        for i in range(NCH):
                            dw = SBD.tile([128, C], f32, name="du")
                            nc.vector.tensor_copy(dw[:], Wt[i][:])
                            nc.sync.dma_start(out=dbg_W[ts(i, 128), :], in_=dw[:])
                            dk = SBD.tile([128, C], f32, name="du")
                            nc.vector.tensor_copy(dk[:], Kn[i][:])
                            nc.sync.dma_start(out=dbg_K[ts(i, 128), :], in_=dk[:])
                        for ci in range(CT):
                            dsb = SBD.tile([128, C], f32, name="du")
                            nc.vector.tensor_copy(dsb[:], S_f32[ci][:])
                            nc.sync.dma_start(out=dbg_S[ts(ci, 128), :], in_=dsb[:])
                            dkv = SBD.tile([128, C], f32, name="du")
                            nc.vector.tensor_copy(dkv[:], KVT[ci][:])
                            nc.sync.dma_start(out=dbg_KVT[ts(ci, 128), :], in_=dkv[:])

    nc.finalize()
    return nc


def _get_nc():
    if "nc" not in _CACHE:
        _CACHE["nc"] = _build()
    return _CACHE["nc"]


def _in_maps(inputs):
    def f(a):
        return np.ascontiguousarray(np.asarray(a, dtype=np.float32))

    x = f(inputs["x"])
    shared = {k: f(inputs[k]) for k in ("Wg", "bg", "Wlr", "blr", "Wo", "bo")}
    return [{"x": x[i], **shared} for i in range(N)]


def _run(in_maps, **kw):
    from concourse.bass_utils import run_bass_kernel_spmd

    nc = _get_nc()
    return run_bass_kernel_spmd(nc, in_maps, list(range(N)), **kw)


def kernel(**inputs) -> np.ndarray:
    res = _run(_in_maps(inputs))
    return np.stack([res.results[i]["out"] for i in range(N)]).astype(np.float32)
